# revision 34
# baseline (speedup 1.0000x reference)
"""GAT+GCN Trainium2 kernel: 8-core SPMD Bass/Tile implementation, v2.

Structure (per core):
  AG(x f16, 5MB) -> Phase B recomputes h[src]/a_src[src] per edge chunk from
  gathered x (contract dim 78), aggregates GAT messages (fh-major layout for
  DVE 2x), computes y = g1 @ gcn_w (f16, y-scale folded) -> y stored fp8,
  AllGathered in 4 row-chunks overlapping B -> Phase D gathers fp8 y,
  aggregates with host-precomputed wsel (sel*norm) tables, pools, MLPs.
  Protein branch (f16) scheduled into the AG(y) window.
"""
import numpy as np
import concourse.bass as bass
import concourse.bacc as bacc
import concourse.mybir as mybir
import concourse.tile as tile

f32 = mybir.dt.float32
f16 = mybir.dt.float16
f8 = mybir.dt.float8e4
i32 = mybir.dt.int32
AF = mybir.ActivationFunctionType
OP = mybir.AluOpType
AX = mybir.AxisListType

F = 78          # input feature dim
H = 10          # heads
HID = 780       # F*H
S_Y = 512.0     # fp8 scale for the y table (folded into gcn_w / fcg1_w)
NQ = 4          # y AllGather row chunks

F8NP = mybir.dt.np(f8)


def ceil_div(a, b):
    return (a + b - 1) // b


def host_prep(inp, n_cores=8):
    """Build per-core input maps + cfg from full inputs."""
    x = np.asarray(inp["x"], np.float32)
    ei = np.asarray(inp["edge_index"], np.int64)
    tgt = np.asarray(inp["target"], np.int64)
    N = x.shape[0]
    B = tgt.shape[0]
    GN = N // B                # nodes per graph
    NS = N // n_cores
    T = NS // 128
    BL = B // n_cores

    loops = np.arange(N, dtype=np.int64)
    src = np.concatenate([ei[0], loops])
    dst = np.concatenate([ei[1], loops])
    E = src.shape[0]

    deg = np.bincount(dst, minlength=N).astype(np.float64)
    dinv = 1.0 / np.sqrt(deg)
    normv = (dinv[src] * dinv[dst]).astype(np.float32)

    order = np.argsort(dst, kind="stable")
    src_s = src[order].astype(np.int32)
    dst_s = dst[order].astype(np.int32)
    norm_s = normv[order]

    gtile = dst_s // 128
    n_gtiles = N // 128
    starts = np.searchsorted(gtile, np.arange(n_gtiles))
    cnts = np.searchsorted(gtile, np.arange(n_gtiles), side="right") - starts
    K = int(np.max(ceil_div(cnts, 128)))

    j = np.arange(E) - starts[gtile]
    eslot = (j % 128).astype(np.int64)
    chunk = (j // 128).astype(np.int64)
    dloc = (dst_s % 128).astype(np.int64)

    srcs_p = np.zeros((n_gtiles, 128, K), np.int32)
    srcs_p[gtile, eslot, chunk] = src_s

    sel_d = np.zeros((n_gtiles, 128, K, 128), np.float16)
    sel_d[gtile, eslot, chunk, dloc] = 1.0
    wsel_d = np.zeros((n_gtiles, 128, K, 128), np.float16)
    wsel_d[gtile, eslot, chunk, dloc] = norm_s
    selT_d = sel_d.transpose(0, 3, 2, 1)          # [gt, d, k, e]

    gat_w = np.asarray(inp["gat_w"], np.float32)          # [F, H*F]
    att_src = np.asarray(inp["att_src"], np.float32)
    att_dst = np.asarray(inp["att_dst"], np.float32)
    As = np.einsum("fhc,hc->fh", gat_w.reshape(F, H, F), att_src)
    Ad = np.einsum("fhc,hc->fh", gat_w.reshape(F, H, F), att_dst)
    asad = np.concatenate([As, Ad], 1).astype(np.float16)     # [F, 2H]
    gatw_fh = np.ascontiguousarray(
        gat_w.reshape(F, H, F).transpose(0, 2, 1).reshape(F, HID)
    ).astype(np.float16)
    gat_b = np.asarray(inp["gat_b"], np.float32)
    gatb_fh = gat_b.reshape(H, F).T.ravel().reshape(1, HID).astype(np.float16)

    gcn_w = np.asarray(inp["gcn_w"], np.float32)
    gcnw_fh = np.ascontiguousarray(
        gcn_w.reshape(H, F, HID).transpose(1, 0, 2).reshape(HID, HID)
    ) * S_Y
    gcnw_fh = gcnw_fh.astype(np.float16)
    gcnbS = (np.asarray(inp["gcn_b"], np.float32) * S_Y).reshape(1, HID)
    gcnbS = gcnbS.astype(np.float16)
    ones1 = np.ones((1, 128), np.float16)

    fcg1_w = np.asarray(inp["fcg1_w"], np.float32).copy()
    fcg1_w *= 1.0 / S_Y
    fcg1_w[HID:] *= 1.0 / GN
    fcg1_w = fcg1_w.astype(np.float16)

    def bias_sw(b, mt):
        b = np.asarray(b, np.float32)
        out = np.zeros((mt * 128,), np.float32)
        out[: b.shape[0]] = b
        return np.ascontiguousarray(out.reshape(mt, 128).T)

    fcg1_bsw = bias_sw(inp["fcg1_b"], 12)
    fcg2_w = np.asarray(inp["fcg2_w"], np.float32)
    fcg2_bsw = bias_sw(inp["fcg2_b"], 1)
    fcg2p = np.zeros((128, 12 * 128), np.float16)
    for kk in range(12):
        kn = min(128, 1500 - kk * 128)
        fcg2p[:kn, kk * 128:kk * 128 + 128] = fcg2_w[kk * 128:kk * 128 + kn]

    convxt_w = np.asarray(inp["convxt_w"], np.float32)
    W2 = np.ascontiguousarray(
        convxt_w.transpose(1, 2, 0).reshape(1000, 8 * 32)).astype(np.float16)
    emb = np.asarray(inp["emb"], np.float32)
    fcxt_w = np.asarray(inp["fcxt_w"], np.float32)
    cb = np.asarray(inp["convxt_b"], np.float32)
    bias_fold = (cb[:, None] * fcxt_w.reshape(32, 121, 128).sum(1)).sum(0)
    fcxt_bsw = bias_sw(np.asarray(inp["fcxt_b"], np.float32) + bias_fold, 1)
    fcxtp = np.zeros((128, 32 * 128), np.float16)
    for o in range(32):
        fcxtp[:121, o * 128:(o + 1) * 128] = fcxt_w[o * 121:(o + 1) * 121]

    fc1_w = np.asarray(inp["fc1_w"], np.float32)
    fc1_bsw = bias_sw(inp["fc1_b"], 8)
    fc1p = np.ascontiguousarray(
        fc1_w.reshape(2, 128, 1024).transpose(1, 0, 2).reshape(128, 2048)
    ).astype(np.float16)
    fc2_w = np.asarray(inp["fc2_w"], np.float32)
    fc2_bsw = bias_sw(inp["fc2_b"], 4)
    fc2p = np.ascontiguousarray(
        fc2_w.reshape(8, 128, 512).transpose(1, 0, 2).reshape(128, 8 * 512)
    ).astype(np.float16)
    out_w = np.asarray(inp["out_w"], np.float32)
    outwp = np.ascontiguousarray(out_w.reshape(4, 128).T).astype(np.float16)
    out_b = np.asarray(inp["out_b"], np.float32).reshape(1, 1)

    iota26 = np.broadcast_to(
        np.tile(np.arange(32, dtype=np.float16), 8), (128, 8 * 32)).copy()
    embp4 = np.zeros((128, 128), np.float16)
    for b4 in range(4):
        embp4[b4 * 32:b4 * 32 + 26, :] = emb
    ng = 128 // GN
    poolm = np.zeros((128, ng), np.float16)
    for g in range(ng):
        poolm[g * GN:(g + 1) * GN, g] = 1.0

    shared = dict(
        asad=asad, gatw_fh=gatw_fh, gatb_fh=gatb_fh, gcnw_fh=gcnw_fh,
        gcnbS=gcnbS, ones1=ones1, fcg1_w=fcg1_w, fcg1_bsw=fcg1_bsw,
        fcg2p=fcg2p, fcg2_bsw=fcg2_bsw, w2=W2, embp4=embp4, iota26=iota26,
        fcxtp=fcxtp, fcxt_bsw=fcxt_bsw, fc1p=fc1p, fc1_bsw=fc1_bsw,
        fc2p=fc2p, fc2_bsw=fc2_bsw, outwp=outwp, out_b=out_b,
        poolm=poolm,
    )

    in_maps = []
    for c in range(n_cores):
        m = dict(shared)
        m["x_sl"] = np.ascontiguousarray(x[c * NS:(c + 1) * NS]).astype(np.float16)
        gt = slice(c * T, (c + 1) * T)
        m["srcs"] = np.ascontiguousarray(
            srcs_p[gt].transpose(1, 0, 2).reshape(128, T * K))
        m["sel_t"] = np.ascontiguousarray(
            sel_d[gt].transpose(1, 0, 2, 3).reshape(128, T * K * 128))
        m["selT_t"] = np.ascontiguousarray(
            selT_d[gt].transpose(1, 0, 2, 3).reshape(128, T * K * 128))
        m["wsel_t"] = np.ascontiguousarray(
            wsel_d[gt].transpose(1, 0, 2, 3).reshape(128, T * K * 128))
        tpad = np.zeros((BL, 1024), np.int64)
        tpad[:, :1000] = tgt[c * BL:(c + 1) * BL]
        tl = tpad.reshape(BL, 8, 128)
        m["t_sb"] = np.ascontiguousarray(
            tl.transpose(2, 0, 1).reshape(128, BL * 8).astype(np.float16))
        in_maps.append(m)

    cfg = dict(n_cores=n_cores, N=N, NS=NS, T=T, BL=BL, K=K, GN=GN)
    return in_maps, cfg


def build(cfg, dbg=False):
    n_cores, NS, T, BL, K, GN = (cfg["n_cores"], cfg["NS"], cfg["T"],
                                 cfg["BL"], cfg["K"], cfg["GN"])
    N = cfg["N"]

    nc = bacc.Bacc(None, target_bir_lowering=False)

    def dinp(name, shape, dt=f32):
        return nc.dram_tensor(name, list(shape), dt, kind="ExternalInput")

    x_sl = dinp("x_sl", (NS, F), f16)
    srcs = dinp("srcs", (128, T * K), i32)
    sel_t = dinp("sel_t", (128, T * K * 128), f16)
    selT_t = dinp("selT_t", (128, T * K * 128), f16)
    wsel_t = dinp("wsel_t", (128, T * K * 128), f16)
    t_sb_d = dinp("t_sb", (128, BL * 8), f16)
    asad_d = dinp("asad", (F, 2 * H), f16)
    gatw_d = dinp("gatw_fh", (F, HID), f16)
    gatb_d = dinp("gatb_fh", (1, HID), f16)
    gcnw_d = dinp("gcnw_fh", (HID, HID), f16)
    gcnb_d = dinp("gcnbS", (1, HID), f16)
    ones1_d = dinp("ones1", (1, 128), f16)
    fcg1_w = dinp("fcg1_w", (2 * HID, 1500), f16)
    fcg1_bsw = dinp("fcg1_bsw", (128, 12))
    fcg2p_d = dinp("fcg2p", (128, 12 * 128), f16)
    fcg2_bsw = dinp("fcg2_bsw", (128, 1))
    w2_d = dinp("w2", (1000, 256), f16)
    embp4_d = dinp("embp4", (128, 128), f16)
    iota26_d = dinp("iota26", (128, 8 * 32), f16)
    fcxtp_d = dinp("fcxtp", (128, 32 * 128), f16)
    fcxt_bsw = dinp("fcxt_bsw", (128, 1))
    fc1p_d = dinp("fc1p", (128, 2048), f16)
    fc1_bsw = dinp("fc1_bsw", (128, 8))
    fc2p_d = dinp("fc2p", (128, 8 * 512), f16)
    fc2_bsw = dinp("fc2_bsw", (128, 4))
    outwp_d = dinp("outwp", (128, 4), f16)
    out_b = dinp("out_b", (1, 1))
    poolm_d = dinp("poolm", (128, 128 // GN), f16)

    outp = nc.dram_tensor("outp", [1, BL], f32, kind="ExternalOutput")
    if dbg:
        o_g1 = nc.dram_tensor("o_g1", [NS, HID], f16, kind="ExternalOutput")
        o_y = nc.dram_tensor("o_y", [NS, HID], f32, kind="ExternalOutput")
        o_g2 = nc.dram_tensor("o_g2", [NS, HID], f16, kind="ExternalOutput")

    x_int = nc.dram_tensor("x_int", [NS, F], f16)
    xtab = nc.dram_tensor("xtab", [N, F], f16, addr_space="Shared")
    y_sl = nc.dram_tensor("y_sl", [NS, HID], f8)
    ytab = nc.dram_tensor("ytab", [N, HID], f8, addr_space="Shared")

    FCH = [(kk * 128, min(128, HID - kk * 128)) for kk in range(ceil_div(HID, 128))]

    def tiles(n, step=128):
        return [(s, min(step, n - s)) for s in range(0, n, step)]

    with tile.TileContext(nc) as tc:
        with (
            tc.tile_pool(name="const", bufs=1) as cpool,
            tc.tile_pool(name="sb", bufs=3) as pool,
            tc.tile_pool(name="w", bufs=2) as wpool,
            tc.tile_pool(name="ps", bufs=2, space="PSUM") as psp,
            tc.tile_pool(name="psg", bufs=1, space="PSUM") as psg,
            tc.tile_pool(name="psm", bufs=2, space="PSUM") as psm,
        ):
            # ============ AllGather x (starts immediately) ============
            nc.sync.dma_start(out=x_int[:], in_=x_sl[:])
            nc.gpsimd.collective_compute(
                "AllGather", OP.bypass,
                replica_groups=[list(range(n_cores))],
                ins=[x_int[:]], outs=[xtab[:]],
            )

            # ---------- resident constants ----------
            def load_const(name, dram, shape, dt=f32):
                t_ = cpool.tile(list(shape), dt, tag=name, name=name)
                nc.sync.dma_start(out=t_[:], in_=dram[:])
                return t_

            asad_sb = load_const("asad", asad_d, [F, 2 * H], f16)
            gatw_sb = load_const("gatw", gatw_d, [F, HID], f16)
            gatb_sb = load_const("gatb", gatb_d, [1, HID], f16)
            gcnb_sb = load_const("gcnb", gcnb_d, [1, HID], f16)
            ones1_sb = load_const("ones1", ones1_d, [1, 128], f16)
            poolm_sb = load_const("poolm", poolm_d, [128, 2], f16)
            t_sb = load_const("tsb", t_sb_d, [128, BL * 8], f16)
            iota26_sb = load_const("iota26", iota26_d, [128, 8 * 32], f16)
            embp4_sb = load_const("embp4", embp4_d, [128, 128], f16)
            fcxtp_sb = load_const("fcxtp", fcxtp_d, [128, 32 * 128], f16)
            fcg1b_sb = load_const("fcg1b", fcg1_bsw, [128, 12])
            fcg2p_sb = load_const("fcg2p", fcg2p_d, [128, 12 * 128], f16)
            fcg2b_sb = load_const("fcg2b", fcg2_bsw, [128, 1])
            fcxtb_sb = load_const("fcxtb", fcxt_bsw, [128, 1])
            fc1p_sb = load_const("fc1p", fc1p_d, [128, 2048], f16)
            fc1b_sb = load_const("fc1b", fc1_bsw, [128, 8])
            fc2p_sb = load_const("fc2p", fc2p_d, [128, 8 * 512], f16)
            fc2b_sb = load_const("fc2b", fc2_bsw, [128, 4])
            outwp_sb = load_const("outwp", outwp_d, [128, 4], f16)
            outb_sb = load_const("outb", out_b, [1, 1])
            srcs_sb = load_const("srcs_all", srcs, [128, T * K], i32)
            selTall_sb = load_const("selT_all", selT_t, [128, T * K * 128], f16)
            gcnw_sb = []
            for kk, (ks, kn) in enumerate(FCH):
                t_ = cpool.tile([128, HID], f16, tag=f"gcnw{kk}", name=f"gcnw{kk}")
                nc.sync.dma_start(out=t_[:kn, :], in_=gcnw_d[ks:ks + kn, :])
                gcnw_sb.append(t_)
            w2_sb = []
            for ic in range(8):
                icn = min(128, 1000 - ic * 128)
                t_ = cpool.tile([128, 256], f16, tag=f"w2{ic}", name=f"w2{ic}")
                nc.sync.dma_start(out=t_[:icn, :], in_=w2_d[ic * 128:ic * 128 + icn, :])
                w2_sb.append(t_)
            adst_sb = cpool.tile([128, T * H], f16, tag="adst")

            # ============ Phase A': a_dst for local nodes ============
            for t in range(T):
                rows = slice(t * 128, (t + 1) * 128)
                x_t = pool.tile([128, 128], f16, tag="x_t")
                nc.sync.dma_start(out=x_t[:, :F], in_=x_sl[rows, :])
                xT = pool.tile([128, 128], f16, tag="xT")
                nc.sync.dma_start_transpose(out=xT[:], in_=x_t[:])
                ad_ps = psm.tile([128, 512], f32, tag="mlp")
                nc.tensor.matmul(out=ad_ps[:, :H], lhsT=xT[:F, :],
                                 rhs=asad_sb[:, H:2 * H], start=True, stop=True)
                nc.vector.tensor_copy(out=adst_sb[:, t * H:(t + 1) * H],
                                      in_=ad_ps[:, :H])

            # ============ Phase B': GAT + y per tile ============
            for t in range(T):
                rows = slice(t * 128, (t + 1) * 128)
                cols = slice(t * K * 128, (t + 1) * K * 128)
                xg = pool.tile([128, K * 128], f16, tag="xg")
                sel_sb = wpool.tile([128, K * 128], f16, tag="sel")
                nc.sync.dma_start(out=sel_sb[:], in_=sel_t[:, cols])
                for c in range(K):
                    nc.gpsimd.indirect_dma_start(
                        out=xg[:, c * 128:c * 128 + F],
                        out_offset=None, in_=xtab[:],
                        in_offset=bass.IndirectOffsetOnAxis(
                            ap=srcs_sb[:, t * K + c:t * K + c + 1], axis=0),
                    )

                # pass 1: transposed x chunks, a_src, a_dst, denominators
                asm_ps = psm.tile([128, 512], f32, tag="mlp")
                xcT = pool.tile([128, K * 128], f16, tag="xcT")
                nc.sync.dma_start_transpose(
                    out=xcT[:].rearrange("p (k d) -> p k d", d=128), in_=xg[:])
                for c in range(K):
                    nc.tensor.matmul(out=asm_ps[:, c * H:(c + 1) * H],
                                     lhsT=xcT[:F, c * 128:(c + 1) * 128],
                                     rhs=asad_sb[:, :H],
                                     start=True, stop=False)
                    nc.tensor.matmul(out=asm_ps[:, c * H:(c + 1) * H],
                                     lhsT=selTall_sb[:, t * K * 128 + c * 128:t * K * 128 + (c + 1) * 128],
                                     rhs=adst_sb[:, t * H:(t + 1) * H],
                                     start=False, stop=True)
                al2 = pool.tile([128, K * H], f32, tag="al2")
                nc.vector.tensor_scalar(out=al2[:], in0=asm_ps[:, :K * H],
                                        scalar1=0.2, scalar2=None, op0=OP.mult)
                nc.vector.tensor_tensor(out=al2[:], in0=al2[:],
                                        in1=asm_ps[:, :K * H], op=OP.max)
                p16 = pool.tile([128, K * H], f16, tag="p16")
                nc.scalar.activation(out=p16[:], in_=al2[:], func=AF.Exp)
                for c in range(K):
                    nc.tensor.matmul(out=asm_ps[:, 192:192 + H],
                                     lhsT=sel_sb[:, c * 128:(c + 1) * 128],
                                     rhs=p16[:, c * H:(c + 1) * H],
                                     start=(c == 0), stop=(c == K - 1))
                rd = pool.tile([128, H], f16, tag="rd")
                with nc.allow_low_precision(reason="rd f16 as matmul rhs"):
                    nc.vector.reciprocal(out=rd[:], in_=asm_ps[:, 192:192 + H])

                # per-edge 1/denom via selT gather-matmul; fold into p16
                rde_ps = psm.tile([128, 512], f32, tag="mlp")
                for c in range(K):
                    nc.tensor.matmul(out=rde_ps[:, c * H:(c + 1) * H],
                                     lhsT=selTall_sb[:, t * K * 128 + c * 128:t * K * 128 + (c + 1) * 128],
                                     rhs=rd[:], start=True, stop=True)
                p16q = pool.tile([128, K * H], f16, tag="p16q")
                nc.vector.tensor_tensor(out=p16q[:], in0=p16[:],
                                        in1=rde_ps[:, :K * H], op=OP.mult)

                # pass 2: h recompute, messages, normalized aggregation
                h_tiles = [None] * K

                def emit_h(c):
                    hp = psp.tile([128, HID], f32, tag="h")
                    nc.tensor.matmul(out=hp[:, :512],
                                     lhsT=xcT[:F, c * 128:(c + 1) * 128],
                                     rhs=gatw_sb[:, :512], start=True, stop=True)
                    nc.tensor.matmul(out=hp[:, 512:],
                                     lhsT=xcT[:F, c * 128:(c + 1) * 128],
                                     rhs=gatw_sb[:, 512:], start=True, stop=True)
                    h_tiles[c] = hp

                emit_h(0)
                emit_h(1)
                g1_ps = psg.tile([128, HID], f32, tag="g1")
                nc.tensor.matmul(out=g1_ps[:, :512], lhsT=ones1_sb[:],
                                 rhs=gatb_sb[:, :512], start=True, stop=False)
                nc.tensor.matmul(out=g1_ps[:, 512:], lhsT=ones1_sb[:],
                                 rhs=gatb_sb[:, 512:], start=True, stop=False)
                for c in range(K):
                    if c + 2 < K:
                        emit_h(c + 2)
                    h_ps = h_tiles[c]
                    m = pool.tile([128, HID], f16, tag=f"m{c % 2}")
                    if c % 2 == 0:
                        nc.vector.tensor_tensor(
                            out=m[:].rearrange("p (f h) -> p f h", h=H),
                            in0=h_ps[:].rearrange("p (f h) -> p f h", h=H),
                            in1=p16q[:, c * H:(c + 1) * H].unsqueeze(1)
                                .to_broadcast([128, F, H]),
                            op=OP.mult)
                    else:
                        h16 = pool.tile([128, HID], f16, tag=f"h16{c % 2}")
                        nc.scalar.activation(out=h16[:], in_=h_ps[:], func=AF.Copy)
                        nc.vector.tensor_tensor(
                            out=m[:].rearrange("p (f h) -> p f h", h=H),
                            in0=h16[:].rearrange("p (f h) -> p f h", h=H),
                            in1=p16q[:, c * H:(c + 1) * H].unsqueeze(1)
                                .to_broadcast([128, F, H]),
                            op=OP.mult)
                    selc = sel_sb[:, c * 128:(c + 1) * 128]
                    nc.tensor.matmul(out=g1_ps[:, :512], lhsT=selc,
                                     rhs=m[:, :512],
                                     start=False, stop=(c == K - 1))
                    nc.tensor.matmul(out=g1_ps[:, 512:], lhsT=selc,
                                     rhs=m[:, 512:],
                                     start=False, stop=(c == K - 1))
                g1t = pool.tile([128, 7 * 128], f16, tag="g1t")
                nc.scalar.activation(out=g1t[:, :HID], in_=g1_ps[:], func=AF.Relu)
                if dbg:
                    nc.sync.dma_start(out=o_g1[rows, :], in_=g1t[:, :HID])

                # y = g1 @ gcn_w  (f16, S_Y folded into gcnw)
                y_ps = psg.tile([128, HID], f32, tag="g1")
                gT = pool.tile([128, 7 * 128], f16, tag="gT")
                nc.sync.dma_start_transpose(
                    out=gT[:].rearrange("p (k d) -> p k d", d=128), in_=g1t[:])
                for kk, (ks, kn) in enumerate(FCH):
                    nc.tensor.matmul(out=y_ps[:, :512],
                                     lhsT=gT[:kn, kk * 128:(kk + 1) * 128],
                                     rhs=gcnw_sb[kk][:kn, :512],
                                     start=(kk == 0), stop=(kk == len(FCH) - 1))
                    nc.tensor.matmul(out=y_ps[:, 512:],
                                     lhsT=gT[:kn, kk * 128:(kk + 1) * 128],
                                     rhs=gcnw_sb[kk][:kn, 512:],
                                     start=(kk == 0), stop=(kk == len(FCH) - 1))
                if dbg:
                    ydbg = pool.tile([128, HID], f32, tag="ydbg")
                    nc.scalar.activation(out=ydbg[:], in_=y_ps[:], func=AF.Copy)
                    nc.sync.dma_start(out=o_y[rows, :], in_=ydbg[:])
                y8 = pool.tile([128, HID], f8, tag="y8")
                nc.scalar.activation(out=y8[:], in_=y_ps[:], func=AF.Copy)
                nc.sync.dma_start(out=y_sl[rows, :], in_=y8[:])

            nc.gpsimd.collective_compute(
                "AllGather", OP.bypass,
                replica_groups=[list(range(n_cores))],
                ins=[y_sl[:]], outs=[ytab[:]],
            )

            # ============ Protein branch (fills AG(y) window) ============
            cvT_sb = cpool.tile([121, BL * 32], f16, tag="cvT")
            for g in range(BL // 2):
                at_ps = psm.tile([128, 512], f32, tag="mlp")
                for j in range(2):
                    b = 2 * g + j
                    oh = pool.tile([128, 8 * 32], f16, tag=f"oh{j}")
                    nc.vector.tensor_tensor(
                        out=oh[:].rearrange("p (k c) -> p k c", c=32),
                        in0=iota26_sb[:].rearrange("p (k c) -> p k c", c=32),
                        in1=t_sb[:, b * 8:(b + 1) * 8].unsqueeze(2)
                            .to_broadcast([128, 8, 32]),
                        op=OP.is_equal)
                    for ic in range(8):
                        icn = min(128, 1000 - ic * 128)
                        nc.tensor.matmul(out=at_ps[:32, 256 * j:256 * j + 256],
                                         lhsT=oh[:icn, ic * 32:(ic + 1) * 32],
                                         rhs=w2_sb[ic][:icn, :],
                                         start=(ic == 0), stop=(ic == 7))
                at_sb = pool.tile([32, 512], f16, tag="at_sb")
                nc.scalar.activation(out=at_sb[:], in_=at_ps[:32, :], func=AF.Copy)
                cv_ps = psm.tile([128, 512], f32, tag="mlp")
                for j in range(2):
                    for k in range(8):
                        nc.tensor.matmul(
                            out=cv_ps[:121, 32 * j:32 * j + 32],
                            lhsT=embp4_sb[:32, k:k + 121],
                            rhs=at_sb[:32, 256 * j + k * 32:256 * j + (k + 1) * 32],
                            start=(k == 0), stop=(k == 7))
                nc.scalar.activation(out=cvT_sb[:, g * 64:(g + 1) * 64],
                                     in_=cv_ps[:121, :64], func=AF.Copy)
            xc1 = cpool.tile([128, BL], f16, tag="xc1")
            xt_ps = psm.tile([128, 512], f32, tag="mlp")
            for o in range(32):
                nc.tensor.matmul(
                    out=xt_ps[:, :BL], lhsT=fcxtp_sb[:121, o * 128:(o + 1) * 128],
                    rhs=cvT_sb[:].rearrange("p (b o) -> p b o", o=32)[:, :, o],
                    start=(o == 0), stop=(o == 31))
            nc.vector.tensor_scalar(out=xc1[:], in0=xt_ps[:, :BL],
                                    scalar1=fcxtb_sb[:, 0:1], scalar2=None,
                                    op0=OP.add)

            # ============ Phase D: GCN aggregation + pooling ============
            gap_acc = cpool.tile([BL, 7 * 128], f16, tag="gap_acc")
            gmpT_sb = []
            gapT_sb = []
            for kk, (ks, kn) in enumerate(FCH):
                gmpT_sb.append(cpool.tile([128, BL], f16, tag=f"gmpT{kk}",
                                          name=f"gmpT{kk}"))
                gapT_sb.append(cpool.tile([128, BL], f16, tag=f"gapT{kk}",
                                          name=f"gapT{kk}"))
            ng = 128 // GN
            for t in range(T):
                rows = slice(t * 128, (t + 1) * 128)
                cols = slice(t * K * 128, (t + 1) * K * 128)
                yg = pool.tile([128, K * HID], f8, tag="yg")
                for c in range(K):
                    nc.gpsimd.indirect_dma_start(
                        out=yg[:, c * HID:(c + 1) * HID],
                        out_offset=None, in_=ytab[:],
                        in_offset=bass.IndirectOffsetOnAxis(
                            ap=srcs_sb[:, t * K + c:t * K + c + 1], axis=0),
                    )
                wsel_sb = wpool.tile([128, K * 128], f16, tag="wsel")
                nc.sync.dma_start(out=wsel_sb[:], in_=wsel_t[:, cols])
                agg_ps = psg.tile([128, HID], f32, tag="g1")
                nc.tensor.matmul(out=agg_ps[:, :512], lhsT=ones1_sb[:],
                                 rhs=gcnb_sb[:, :512], start=True, stop=False)
                nc.tensor.matmul(out=agg_ps[:, 512:], lhsT=ones1_sb[:],
                                 rhs=gcnb_sb[:, 512:], start=True, stop=False)
                for c in range(K):
                    nc.tensor.matmul(out=agg_ps[:, :512],
                                     lhsT=wsel_sb[:, c * 128:(c + 1) * 128],
                                     rhs=yg[:, c * HID:c * HID + 512],
                                     start=False, stop=(c == K - 1))
                    nc.tensor.matmul(out=agg_ps[:, 512:],
                                     lhsT=wsel_sb[:, c * 128:(c + 1) * 128],
                                     rhs=yg[:, c * HID + 512:(c + 1) * HID],
                                     start=False, stop=(c == K - 1))
                g2b = pool.tile([128, 7 * 128], f16, tag="g2b")
                nc.scalar.activation(out=g2b[:, :HID], in_=agg_ps[:], func=AF.Relu)
                if dbg:
                    nc.sync.dma_start(out=o_g2[rows, :], in_=g2b[:, :HID])
                gp_ps = psg.tile([128, HID], f32, tag="g1")
                nc.tensor.matmul(out=gp_ps[:ng, :512], lhsT=poolm_sb[:, :ng],
                                 rhs=g2b[:, :512], start=True, stop=True)
                nc.tensor.matmul(out=gp_ps[:ng, 512:HID], lhsT=poolm_sb[:, :ng],
                                 rhs=g2b[:, 512:HID], start=True, stop=True)
                gtmp = pool.tile([ng, HID], f16, tag="gtmp")
                nc.scalar.activation(out=gtmp[:], in_=gp_ps[:ng, :HID], func=AF.Copy)
                nc.sync.dma_start(out=gap_acc[ng * t:ng * (t + 1), :HID], in_=gtmp[:])
                tpT = pool.tile([128, 7 * 128], f16, tag="tpT")
                nc.sync.dma_start_transpose(
                    out=tpT[:].rearrange("p (k d) -> p k d", d=128), in_=g2b[:])
                for kk, (ks, kn) in enumerate(FCH):
                    nc.vector.reduce_max(
                        out=gmpT_sb[kk][:kn, ng * t:ng * (t + 1)],
                        in_=tpT[:kn, kk * 128:(kk + 1) * 128]
                            .rearrange("p (g n) -> p g n", n=GN),
                        axis=AX.X)

            gtT = pool.tile([128, 7 * BL], f16, tag="gtT")
            nc.sync.dma_start_transpose(
                out=gtT[:].rearrange("p (k b) -> p k b", b=BL), in_=gap_acc[:])
            for kk, (ks, kn) in enumerate(FCH):
                nc.vector.tensor_copy(out=gapT_sb[kk][:kn, :],
                                      in_=gtT[:kn, kk * BL:(kk + 1) * BL])

            # ============ MLPs ============
            y1_sb = cpool.tile([128, 12 * BL], f16, tag="y1")
            mtiles = tiles(1500)
            for mj in range(0, len(mtiles), 2):
                y_ps = psm.tile([128, 512], f32, tag="mlp")
                for pj, (ms, mn) in enumerate(mtiles[mj:mj + 2]):
                    mi = mj + pj
                    col = pj * BL
                    wt_a = wpool.tile([128, 6 * 128], f16, tag=f"wta{pj}")
                    nc.sync.dma_start(
                        out=wt_a[:, :6 * mn].rearrange("p (k m) -> p k m", m=mn),
                        in_=fcg1_w[0:768, ms:ms + mn]
                            .rearrange("(k p) m -> p k m", p=128))
                    wt_b = wpool.tile([128, 128], f16, tag=f"wtb{pj}")
                    nc.sync.dma_start(out=wt_b[:12, :mn],
                                      in_=fcg1_w[768:780, ms:ms + mn])
                    wt_c = wpool.tile([128, 6 * 128], f16, tag=f"wtc{pj}")
                    nc.sync.dma_start(
                        out=wt_c[:, :6 * mn].rearrange("p (k m) -> p k m", m=mn),
                        in_=fcg1_w[HID:HID + 768, ms:ms + mn]
                            .rearrange("(k p) m -> p k m", p=128))
                    wt_d = wpool.tile([128, 128], f16, tag=f"wtd{pj}")
                    nc.sync.dma_start(out=wt_d[:12, :mn],
                                      in_=fcg1_w[HID + 768:HID + 780, ms:ms + mn])
                    for kk, (ks, kn) in enumerate(FCH):
                        lhsT = (wt_a[:, kk * mn:(kk + 1) * mn] if kk < 6
                                else wt_b[:12, :mn])
                        nc.tensor.matmul(out=y_ps[:mn, col:col + BL], lhsT=lhsT,
                                         rhs=gmpT_sb[kk][:kn, :],
                                         start=(kk == 0), stop=False)
                    for kk, (ks, kn) in enumerate(FCH):
                        lhsT = (wt_c[:, kk * mn:(kk + 1) * mn] if kk < 6
                                else wt_d[:12, :mn])
                        nc.tensor.matmul(out=y_ps[:mn, col:col + BL], lhsT=lhsT,
                                         rhs=gapT_sb[kk][:kn, :],
                                         start=False, stop=(kk == len(FCH) - 1))
                    nc.scalar.activation(out=y1_sb[:mn, mi * BL:(mi + 1) * BL],
                                         in_=y_ps[:mn, col:col + BL], func=AF.Relu,
                                         bias=fcg1b_sb[:mn, mi:mi + 1])

            xc0 = cpool.tile([128, BL], f16, tag="xc0")
            y2_ps = psm.tile([128, 512], f32, tag="mlp")
            kt2 = tiles(1500)
            for kk, (ks, kn) in enumerate(kt2):
                nc.tensor.matmul(out=y2_ps[:, :BL],
                                 lhsT=fcg2p_sb[:kn, kk * 128:kk * 128 + 128],
                                 rhs=y1_sb[:kn, kk * BL:(kk + 1) * BL],
                                 start=(kk == 0), stop=(kk == len(kt2) - 1))
            nc.vector.tensor_scalar(out=xc0[:], in0=y2_ps[:, :BL],
                                    scalar1=fcg2b_sb[:, 0:1], scalar2=None,
                                    op0=OP.add)

            # ---- head ----
            y3_sb = cpool.tile([128, 8 * BL], f16, tag="y3")
            for mi in range(8):
                y_ps = psm.tile([128, 512], f32, tag="mlp")
                for kk in range(2):
                    rhs = xc0 if kk == 0 else xc1
                    nc.tensor.matmul(
                        out=y_ps[:, :BL],
                        lhsT=fc1p_sb[:, kk * 1024 + mi * 128:
                                     kk * 1024 + (mi + 1) * 128],
                        rhs=rhs[:], start=(kk == 0), stop=(kk == 1))
                nc.scalar.activation(out=y3_sb[:, mi * BL:(mi + 1) * BL],
                                     in_=y_ps[:, :BL],
                                     func=AF.Relu, bias=fc1b_sb[:, mi:mi + 1])
            y4_sb = cpool.tile([128, 4 * BL], f16, tag="y4")
            for mi in range(4):
                y_ps = psm.tile([128, 512], f32, tag="mlp")
                for kk in range(8):
                    nc.tensor.matmul(out=y_ps[:, :BL],
                                     lhsT=fc2p_sb[:, kk * 512 + mi * 128:
                                                  kk * 512 + (mi + 1) * 128],
                                     rhs=y3_sb[:, kk * BL:(kk + 1) * BL],
                                     start=(kk == 0), stop=(kk == 7))
                nc.scalar.activation(out=y4_sb[:, mi * BL:(mi + 1) * BL],
                                     in_=y_ps[:, :BL],
                                     func=AF.Relu, bias=fc2b_sb[:, mi:mi + 1])
            o_ps = psm.tile([128, 512], f32, tag="mlp")
            for kk in range(4):
                nc.tensor.matmul(out=o_ps[:1, :BL], lhsT=outwp_sb[:, kk:kk + 1],
                                 rhs=y4_sb[:, kk * BL:(kk + 1) * BL],
                                 start=(kk == 0), stop=(kk == 3))
            o_sb = cpool.tile([1, BL], f32, tag="o_sb")
            nc.vector.tensor_scalar(out=o_sb[:], in0=o_ps[:1, :BL],
                                    scalar1=outb_sb[:, 0:1], scalar2=None,
                                    op0=OP.add)
            nc.sync.dma_start(out=outp[:], in_=o_sb[:])

    nc.finalize()
    return nc


def run(inp, n_cores=8, trace=False, dbg=False):
    from concourse.bass_utils import run_bass_kernel_spmd
    in_maps, cfg = host_prep(inp, n_cores)
    nc = build(cfg, dbg=dbg)
    res = run_bass_kernel_spmd(nc, in_maps, list(range(n_cores)))
    out = np.concatenate(
        [res.results[c]["outp"].reshape(-1, 1) for c in range(n_cores)], 0)
    return out, res


_CACHED = {}


def kernel(**inputs):
    """Full-input entry point: shards across 8 NeuronCores internally."""
    n_cores = 8
    in_maps, cfg = host_prep(inputs, n_cores)
    key = (cfg["N"], cfg["T"], cfg["BL"], cfg["K"], cfg["GN"])
    nc = _CACHED.get(key)
    if nc is None:
        nc = build(cfg)
        _CACHED[key] = nc
    from concourse.bass_utils import run_bass_kernel_spmd
    res = run_bass_kernel_spmd(nc, in_maps, list(range(n_cores)))
    out = np.concatenate(
        [res.results[c]["outp"].reshape(-1, 1) for c in range(n_cores)], 0)
    return out.astype(np.float32)


# revision 41
# speedup vs baseline: 1.0910x; 1.0910x over previous
"""GAT+GCN Trainium2 kernel: 8-core SPMD Bass/Tile implementation, v2.

Structure (per core):
  AG(x f16, 5MB) -> Phase B recomputes h[src]/a_src[src] per edge chunk from
  gathered x (contract dim 78), aggregates GAT messages (fh-major layout for
  DVE 2x), computes y = g1 @ gcn_w (f16, y-scale folded) -> y stored fp8,
  AllGathered in 4 row-chunks overlapping B -> Phase D gathers fp8 y,
  aggregates with host-precomputed wsel (sel*norm) tables, pools, MLPs.
  Protein branch (f16) scheduled into the AG(y) window.
"""
import numpy as np
import concourse.bass as bass
import concourse.bacc as bacc
import concourse.mybir as mybir
import concourse.tile as tile

f32 = mybir.dt.float32
f16 = mybir.dt.float16
f8 = mybir.dt.float8e4
i32 = mybir.dt.int32
AF = mybir.ActivationFunctionType
OP = mybir.AluOpType
AX = mybir.AxisListType

F = 78          # input feature dim
H = 10          # heads
HID = 780       # F*H
S_Y = 512.0     # fp8 scale for the y table (folded into gcn_w / fcg1_w)
NQ = 4          # y AllGather row chunks

F8NP = mybir.dt.np(f8)


def ceil_div(a, b):
    return (a + b - 1) // b


def host_prep(inp, n_cores=8):
    """Build per-core input maps + cfg from full inputs."""
    x = np.asarray(inp["x"], np.float32)
    ei = np.asarray(inp["edge_index"], np.int64)
    tgt = np.asarray(inp["target"], np.int64)
    N = x.shape[0]
    B = tgt.shape[0]
    GN = N // B                # nodes per graph
    NS = N // n_cores
    T = NS // 128
    BL = B // n_cores

    loops = np.arange(N, dtype=np.int64)
    src = np.concatenate([ei[0], loops])
    dst = np.concatenate([ei[1], loops])
    E = src.shape[0]

    deg = np.bincount(dst, minlength=N).astype(np.float64)
    dinv = 1.0 / np.sqrt(deg)
    normv = (dinv[src] * dinv[dst]).astype(np.float32)

    order = np.argsort(dst, kind="stable")
    src_s = src[order].astype(np.int32)
    dst_s = dst[order].astype(np.int32)
    norm_s = normv[order]

    gtile = dst_s // 128
    n_gtiles = N // 128
    starts = np.searchsorted(gtile, np.arange(n_gtiles))
    cnts = np.searchsorted(gtile, np.arange(n_gtiles), side="right") - starts
    K = int(np.max(ceil_div(cnts, 128)))

    j = np.arange(E) - starts[gtile]
    eslot = (j % 128).astype(np.int64)
    chunk = (j // 128).astype(np.int64)
    dloc = (dst_s % 128).astype(np.int64)

    srcs_p = np.zeros((n_gtiles, 128, K), np.int32)
    srcs_p[gtile, eslot, chunk] = src_s

    sel_d = np.zeros((n_gtiles, 128, K, 128), np.float16)
    sel_d[gtile, eslot, chunk, dloc] = 1.0
    wsel_d = np.zeros((n_gtiles, 128, K, 128), np.float16)
    wsel_d[gtile, eslot, chunk, dloc] = norm_s
    selT_d = sel_d.transpose(0, 3, 2, 1)          # [gt, d, k, e]

    gat_w = np.asarray(inp["gat_w"], np.float32)          # [F, H*F]
    att_src = np.asarray(inp["att_src"], np.float32)
    att_dst = np.asarray(inp["att_dst"], np.float32)
    As = np.einsum("fhc,hc->fh", gat_w.reshape(F, H, F), att_src)
    Ad = np.einsum("fhc,hc->fh", gat_w.reshape(F, H, F), att_dst)
    asad = np.concatenate([As, Ad], 1).astype(np.float16)     # [F, 2H]
    gatw_fh = np.ascontiguousarray(
        gat_w.reshape(F, H, F).transpose(0, 2, 1).reshape(F, HID)
    ).astype(np.float16)
    gat_b = np.asarray(inp["gat_b"], np.float32)
    gatb_fh = gat_b.reshape(H, F).T.ravel().reshape(1, HID).astype(np.float16)

    gcn_w = np.asarray(inp["gcn_w"], np.float32)
    gcnw_fh = np.ascontiguousarray(
        gcn_w.reshape(H, F, HID).transpose(1, 0, 2).reshape(HID, HID)
    ) * S_Y
    gcnw_fh = gcnw_fh.astype(np.float16)
    gcnbS = (np.asarray(inp["gcn_b"], np.float32) * S_Y).reshape(1, HID)
    gcnbS = gcnbS.astype(np.float16)
    ones1 = np.ones((1, 128), np.float16)

    fcg1_w = np.asarray(inp["fcg1_w"], np.float32).copy()
    fcg1_w *= 1.0 / S_Y
    fcg1_w[HID:] *= 1.0 / GN
    fcg1_w = fcg1_w.astype(np.float16)

    def bias_sw(b, mt):
        b = np.asarray(b, np.float32)
        out = np.zeros((mt * 128,), np.float32)
        out[: b.shape[0]] = b
        return np.ascontiguousarray(out.reshape(mt, 128).T)

    fcg1_bsw = bias_sw(inp["fcg1_b"], 12)
    fcg2_w = np.asarray(inp["fcg2_w"], np.float32)
    fcg2_bsw = bias_sw(inp["fcg2_b"], 1)
    fcg2p = np.zeros((128, 12 * 128), np.float16)
    for kk in range(12):
        kn = min(128, 1500 - kk * 128)
        fcg2p[:kn, kk * 128:kk * 128 + 128] = fcg2_w[kk * 128:kk * 128 + kn]

    convxt_w = np.asarray(inp["convxt_w"], np.float32)
    W2 = np.ascontiguousarray(
        convxt_w.transpose(1, 2, 0).reshape(1000, 8 * 32)).astype(np.float16)
    emb = np.asarray(inp["emb"], np.float32)
    fcxt_w = np.asarray(inp["fcxt_w"], np.float32)
    cb = np.asarray(inp["convxt_b"], np.float32)
    bias_fold = (cb[:, None] * fcxt_w.reshape(32, 121, 128).sum(1)).sum(0)
    fcxt_bsw = bias_sw(np.asarray(inp["fcxt_b"], np.float32) + bias_fold, 1)
    fcxtp = np.zeros((128, 32 * 128), np.float16)
    for o in range(32):
        fcxtp[:121, o * 128:(o + 1) * 128] = fcxt_w[o * 121:(o + 1) * 121]

    fc1_w = np.asarray(inp["fc1_w"], np.float32)
    fc1_bsw = bias_sw(inp["fc1_b"], 8)
    fc1p = np.ascontiguousarray(
        fc1_w.reshape(2, 128, 1024).transpose(1, 0, 2).reshape(128, 2048)
    ).astype(np.float16)
    fc2_w = np.asarray(inp["fc2_w"], np.float32)
    fc2_bsw = bias_sw(inp["fc2_b"], 4)
    fc2p = np.ascontiguousarray(
        fc2_w.reshape(8, 128, 512).transpose(1, 0, 2).reshape(128, 8 * 512)
    ).astype(np.float16)
    out_w = np.asarray(inp["out_w"], np.float32)
    outwp = np.ascontiguousarray(out_w.reshape(4, 128).T).astype(np.float16)
    out_b = np.asarray(inp["out_b"], np.float32).reshape(1, 1)

    iota26 = np.broadcast_to(
        np.tile(np.arange(32, dtype=np.float16), 8), (128, 8 * 32)).copy()
    embp4 = np.zeros((128, 128), np.float16)
    for b4 in range(4):
        embp4[b4 * 32:b4 * 32 + 26, :] = emb
    ng = 128 // GN
    poolm = np.zeros((128, ng), np.float16)
    for g in range(ng):
        poolm[g * GN:(g + 1) * GN, g] = 1.0

    shared = dict(
        asad=asad, gatw_fh=gatw_fh, gatb_fh=gatb_fh, gcnw_fh=gcnw_fh,
        gcnbS=gcnbS, ones1=ones1, fcg1_w=fcg1_w, fcg1_bsw=fcg1_bsw,
        fcg2p=fcg2p, fcg2_bsw=fcg2_bsw, w2=W2, embp4=embp4, iota26=iota26,
        fcxtp=fcxtp, fcxt_bsw=fcxt_bsw, fc1p=fc1p, fc1_bsw=fc1_bsw,
        fc2p=fc2p, fc2_bsw=fc2_bsw, outwp=outwp, out_b=out_b,
        poolm=poolm,
    )

    x16 = np.ascontiguousarray(x).astype(np.float16)
    in_maps = []
    for c in range(n_cores):
        m = dict(shared)
        m["x_full"] = x16
        m["x_sl"] = np.ascontiguousarray(x[c * NS:(c + 1) * NS]).astype(np.float16)
        gt = slice(c * T, (c + 1) * T)
        m["srcs"] = np.ascontiguousarray(
            srcs_p[gt].transpose(1, 0, 2).reshape(128, T * K))
        m["sel_t"] = np.ascontiguousarray(
            sel_d[gt].transpose(1, 0, 2, 3).reshape(128, T * K * 128))
        m["selT_t"] = np.ascontiguousarray(
            selT_d[gt].transpose(1, 0, 2, 3).reshape(128, T * K * 128))
        m["wsel_t"] = np.ascontiguousarray(
            wsel_d[gt].transpose(1, 0, 2, 3).reshape(128, T * K * 128))
        tpad = np.zeros((BL, 1024), np.int64)
        tpad[:, :1000] = tgt[c * BL:(c + 1) * BL]
        tl = tpad.reshape(BL, 8, 128)
        m["t_sb"] = np.ascontiguousarray(
            tl.transpose(2, 0, 1).reshape(128, BL * 8).astype(np.float16))
        in_maps.append(m)

    cfg = dict(n_cores=n_cores, N=N, NS=NS, T=T, BL=BL, K=K, GN=GN)
    return in_maps, cfg


def build(cfg, dbg=False):
    n_cores, NS, T, BL, K, GN = (cfg["n_cores"], cfg["NS"], cfg["T"],
                                 cfg["BL"], cfg["K"], cfg["GN"])
    N = cfg["N"]

    nc = bacc.Bacc(None, target_bir_lowering=False)

    def dinp(name, shape, dt=f32):
        return nc.dram_tensor(name, list(shape), dt, kind="ExternalInput")

    x_sl = dinp("x_sl", (NS, F), f16)
    srcs = dinp("srcs", (128, T * K), i32)
    sel_t = dinp("sel_t", (128, T * K * 128), f16)
    selT_t = dinp("selT_t", (128, T * K * 128), f16)
    wsel_t = dinp("wsel_t", (128, T * K * 128), f16)
    t_sb_d = dinp("t_sb", (128, BL * 8), f16)
    asad_d = dinp("asad", (F, 2 * H), f16)
    gatw_d = dinp("gatw_fh", (F, HID), f16)
    gatb_d = dinp("gatb_fh", (1, HID), f16)
    gcnw_d = dinp("gcnw_fh", (HID, HID), f16)
    gcnb_d = dinp("gcnbS", (1, HID), f16)
    ones1_d = dinp("ones1", (1, 128), f16)
    fcg1_w = dinp("fcg1_w", (2 * HID, 1500), f16)
    fcg1_bsw = dinp("fcg1_bsw", (128, 12))
    fcg2p_d = dinp("fcg2p", (128, 12 * 128), f16)
    fcg2_bsw = dinp("fcg2_bsw", (128, 1))
    w2_d = dinp("w2", (1000, 256), f16)
    embp4_d = dinp("embp4", (128, 128), f16)
    iota26_d = dinp("iota26", (128, 8 * 32), f16)
    fcxtp_d = dinp("fcxtp", (128, 32 * 128), f16)
    fcxt_bsw = dinp("fcxt_bsw", (128, 1))
    fc1p_d = dinp("fc1p", (128, 2048), f16)
    fc1_bsw = dinp("fc1_bsw", (128, 8))
    fc2p_d = dinp("fc2p", (128, 8 * 512), f16)
    fc2_bsw = dinp("fc2_bsw", (128, 4))
    outwp_d = dinp("outwp", (128, 4), f16)
    out_b = dinp("out_b", (1, 1))
    poolm_d = dinp("poolm", (128, 128 // GN), f16)

    outp = nc.dram_tensor("outp", [1, BL], f32, kind="ExternalOutput")
    if dbg:
        o_g1 = nc.dram_tensor("o_g1", [NS, HID], f16, kind="ExternalOutput")
        o_y = nc.dram_tensor("o_y", [NS, HID], f32, kind="ExternalOutput")
        o_g2 = nc.dram_tensor("o_g2", [NS, HID], f16, kind="ExternalOutput")

    xtab = dinp("x_full", (N, F), f16)
    y_sl = nc.dram_tensor("y_sl", [NS, HID], f8)
    ytab = nc.dram_tensor("ytab", [N, HID], f8, addr_space="Shared")

    FCH = [(kk * 128, min(128, HID - kk * 128)) for kk in range(ceil_div(HID, 128))]

    def tiles(n, step=128):
        return [(s, min(step, n - s)) for s in range(0, n, step)]

    with tile.TileContext(nc) as tc:
        with (
            tc.tile_pool(name="const", bufs=1) as cpool,
            tc.tile_pool(name="sb", bufs=3) as pool,
            tc.tile_pool(name="w", bufs=2) as wpool,
            tc.tile_pool(name="ps", bufs=2, space="PSUM") as psp,
            tc.tile_pool(name="psg", bufs=1, space="PSUM") as psg,
            tc.tile_pool(name="psm", bufs=2, space="PSUM") as psm,
        ):
            # ---------- resident constants ----------
            def load_const(name, dram, shape, dt=f32):
                t_ = cpool.tile(list(shape), dt, tag=name, name=name)
                nc.sync.dma_start(out=t_[:], in_=dram[:])
                return t_

            asad_sb = load_const("asad", asad_d, [F, 2 * H], f16)
            gatw_sb = load_const("gatw", gatw_d, [F, HID], f16)
            gatb_sb = load_const("gatb", gatb_d, [1, HID], f16)
            gcnb_sb = load_const("gcnb", gcnb_d, [1, HID], f16)
            ones1_sb = load_const("ones1", ones1_d, [1, 128], f16)
            poolm_sb = load_const("poolm", poolm_d, [128, 2], f16)
            t_sb = load_const("tsb", t_sb_d, [128, BL * 8], f16)
            iota26_sb = load_const("iota26", iota26_d, [128, 8 * 32], f16)
            embp4_sb = load_const("embp4", embp4_d, [128, 128], f16)
            fcxtp_sb = load_const("fcxtp", fcxtp_d, [128, 32 * 128], f16)
            fcg1b_sb = load_const("fcg1b", fcg1_bsw, [128, 12])
            fcg2p_sb = load_const("fcg2p", fcg2p_d, [128, 12 * 128], f16)
            fcg2b_sb = load_const("fcg2b", fcg2_bsw, [128, 1])
            fcxtb_sb = load_const("fcxtb", fcxt_bsw, [128, 1])
            fc1p_sb = load_const("fc1p", fc1p_d, [128, 2048], f16)
            fc1b_sb = load_const("fc1b", fc1_bsw, [128, 8])
            fc2p_sb = load_const("fc2p", fc2p_d, [128, 8 * 512], f16)
            fc2b_sb = load_const("fc2b", fc2_bsw, [128, 4])
            outwp_sb = load_const("outwp", outwp_d, [128, 4], f16)
            outb_sb = load_const("outb", out_b, [1, 1])
            srcs_sb = load_const("srcs_all", srcs, [128, T * K], i32)
            selTall_sb = load_const("selT_all", selT_t, [128, T * K * 128], f16)
            gcnw_sb = []
            for kk, (ks, kn) in enumerate(FCH):
                t_ = cpool.tile([128, HID], f16, tag=f"gcnw{kk}", name=f"gcnw{kk}")
                nc.sync.dma_start(out=t_[:kn, :], in_=gcnw_d[ks:ks + kn, :])
                gcnw_sb.append(t_)
            w2_sb = []
            for ic in range(8):
                icn = min(128, 1000 - ic * 128)
                t_ = cpool.tile([128, 256], f16, tag=f"w2{ic}", name=f"w2{ic}")
                nc.sync.dma_start(out=t_[:icn, :], in_=w2_d[ic * 128:ic * 128 + icn, :])
                w2_sb.append(t_)
            adst_sb = cpool.tile([128, T * H], f16, tag="adst")

            # ============ Phase A': a_dst for local nodes ============
            for t in range(T):
                rows = slice(t * 128, (t + 1) * 128)
                x_t = pool.tile([128, 128], f16, tag="x_t")
                nc.sync.dma_start(out=x_t[:, :F], in_=x_sl[rows, :])
                xT = pool.tile([128, 128], f16, tag="xT")
                nc.sync.dma_start_transpose(out=xT[:], in_=x_t[:])
                ad_ps = psm.tile([128, 512], f32, tag="mlp")
                nc.tensor.matmul(out=ad_ps[:, :H], lhsT=xT[:F, :],
                                 rhs=asad_sb[:, H:2 * H], start=True, stop=True)
                nc.vector.tensor_copy(out=adst_sb[:, t * H:(t + 1) * H],
                                      in_=ad_ps[:, :H])

            # ============ Phase B': GAT + y per tile ============
            for t in range(T):
                rows = slice(t * 128, (t + 1) * 128)
                cols = slice(t * K * 128, (t + 1) * K * 128)
                xg = pool.tile([128, K * 128], f16, tag="xg")
                sel_sb = wpool.tile([128, K * 128], f16, tag="sel")
                nc.sync.dma_start(out=sel_sb[:], in_=sel_t[:, cols])
                for c in range(K):
                    nc.gpsimd.indirect_dma_start(
                        out=xg[:, c * 128:c * 128 + F],
                        out_offset=None, in_=xtab[:],
                        in_offset=bass.IndirectOffsetOnAxis(
                            ap=srcs_sb[:, t * K + c:t * K + c + 1], axis=0),
                    )

                # pass 1: transposed x chunks, a_src, a_dst, denominators
                asm_ps = psm.tile([128, 512], f32, tag="mlp")
                xcT = pool.tile([128, K * 128], f16, tag="xcT")
                nc.sync.dma_start_transpose(
                    out=xcT[:].rearrange("p (k d) -> p k d", d=128), in_=xg[:])
                for c in range(K):
                    nc.tensor.matmul(out=asm_ps[:, c * H:(c + 1) * H],
                                     lhsT=xcT[:F, c * 128:(c + 1) * 128],
                                     rhs=asad_sb[:, :H],
                                     start=True, stop=False)
                    nc.tensor.matmul(out=asm_ps[:, c * H:(c + 1) * H],
                                     lhsT=selTall_sb[:, t * K * 128 + c * 128:t * K * 128 + (c + 1) * 128],
                                     rhs=adst_sb[:, t * H:(t + 1) * H],
                                     start=False, stop=True)
                al2 = pool.tile([128, K * H], f32, tag="al2")
                nc.vector.tensor_scalar(out=al2[:], in0=asm_ps[:, :K * H],
                                        scalar1=0.2, scalar2=None, op0=OP.mult)
                nc.vector.tensor_tensor(out=al2[:], in0=al2[:],
                                        in1=asm_ps[:, :K * H], op=OP.max)
                p16 = pool.tile([128, K * H], f16, tag="p16")
                nc.scalar.activation(out=p16[:], in_=al2[:], func=AF.Exp)
                for c in range(K):
                    nc.tensor.matmul(out=asm_ps[:, 192:192 + H],
                                     lhsT=sel_sb[:, c * 128:(c + 1) * 128],
                                     rhs=p16[:, c * H:(c + 1) * H],
                                     start=(c == 0), stop=(c == K - 1))
                rd = pool.tile([128, H], f16, tag="rd")
                with nc.allow_low_precision(reason="rd f16 as matmul rhs"):
                    nc.vector.reciprocal(out=rd[:], in_=asm_ps[:, 192:192 + H])

                # per-edge 1/denom via selT gather-matmul; fold into p16
                rde_ps = psm.tile([128, 512], f32, tag="mlp")
                for c in range(K):
                    nc.tensor.matmul(out=rde_ps[:, c * H:(c + 1) * H],
                                     lhsT=selTall_sb[:, t * K * 128 + c * 128:t * K * 128 + (c + 1) * 128],
                                     rhs=rd[:], start=True, stop=True)
                p16q = pool.tile([128, K * H], f16, tag="p16q")
                nc.vector.tensor_tensor(out=p16q[:], in0=p16[:],
                                        in1=rde_ps[:, :K * H], op=OP.mult)

                # pass 2: h recompute, messages, normalized aggregation
                h_tiles = [None] * K

                def emit_h(c):
                    hp = psp.tile([128, HID], f32, tag="h")
                    nc.tensor.matmul(out=hp[:, :512],
                                     lhsT=xcT[:F, c * 128:(c + 1) * 128],
                                     rhs=gatw_sb[:, :512], start=True, stop=True)
                    nc.tensor.matmul(out=hp[:, 512:],
                                     lhsT=xcT[:F, c * 128:(c + 1) * 128],
                                     rhs=gatw_sb[:, 512:], start=True, stop=True)
                    h_tiles[c] = hp

                emit_h(0)
                emit_h(1)
                g1_ps = psg.tile([128, HID], f32, tag="g1")
                nc.tensor.matmul(out=g1_ps[:, :512], lhsT=ones1_sb[:],
                                 rhs=gatb_sb[:, :512], start=True, stop=False)
                nc.tensor.matmul(out=g1_ps[:, 512:], lhsT=ones1_sb[:],
                                 rhs=gatb_sb[:, 512:], start=True, stop=False)
                for c in range(K):
                    if c + 2 < K:
                        emit_h(c + 2)
                    h_ps = h_tiles[c]
                    m = pool.tile([128, HID], f16, tag=f"m{c % 2}")
                    if c % 2 == 0:
                        nc.vector.tensor_tensor(
                            out=m[:].rearrange("p (f h) -> p f h", h=H),
                            in0=h_ps[:].rearrange("p (f h) -> p f h", h=H),
                            in1=p16q[:, c * H:(c + 1) * H].unsqueeze(1)
                                .to_broadcast([128, F, H]),
                            op=OP.mult)
                    else:
                        h16 = pool.tile([128, HID], f16, tag=f"h16{c % 2}")
                        nc.scalar.activation(out=h16[:], in_=h_ps[:], func=AF.Copy)
                        nc.vector.tensor_tensor(
                            out=m[:].rearrange("p (f h) -> p f h", h=H),
                            in0=h16[:].rearrange("p (f h) -> p f h", h=H),
                            in1=p16q[:, c * H:(c + 1) * H].unsqueeze(1)
                                .to_broadcast([128, F, H]),
                            op=OP.mult)
                    selc = sel_sb[:, c * 128:(c + 1) * 128]
                    nc.tensor.matmul(out=g1_ps[:, :512], lhsT=selc,
                                     rhs=m[:, :512],
                                     start=False, stop=(c == K - 1))
                    nc.tensor.matmul(out=g1_ps[:, 512:], lhsT=selc,
                                     rhs=m[:, 512:],
                                     start=False, stop=(c == K - 1))
                g1t = pool.tile([128, 7 * 128], f16, tag="g1t")
                nc.scalar.activation(out=g1t[:, :HID], in_=g1_ps[:], func=AF.Relu)
                if dbg:
                    nc.sync.dma_start(out=o_g1[rows, :], in_=g1t[:, :HID])

                # y = g1 @ gcn_w  (f16, S_Y folded into gcnw)
                y_ps = psg.tile([128, HID], f32, tag="g1")
                gT = pool.tile([128, 7 * 128], f16, tag="gT")
                nc.sync.dma_start_transpose(
                    out=gT[:].rearrange("p (k d) -> p k d", d=128), in_=g1t[:])
                for kk, (ks, kn) in enumerate(FCH):
                    nc.tensor.matmul(out=y_ps[:, :512],
                                     lhsT=gT[:kn, kk * 128:(kk + 1) * 128],
                                     rhs=gcnw_sb[kk][:kn, :512],
                                     start=(kk == 0), stop=(kk == len(FCH) - 1))
                    nc.tensor.matmul(out=y_ps[:, 512:],
                                     lhsT=gT[:kn, kk * 128:(kk + 1) * 128],
                                     rhs=gcnw_sb[kk][:kn, 512:],
                                     start=(kk == 0), stop=(kk == len(FCH) - 1))
                if dbg:
                    ydbg = pool.tile([128, HID], f32, tag="ydbg")
                    nc.scalar.activation(out=ydbg[:], in_=y_ps[:], func=AF.Copy)
                    nc.sync.dma_start(out=o_y[rows, :], in_=ydbg[:])
                y8 = pool.tile([128, HID], f8, tag="y8")
                nc.scalar.activation(out=y8[:], in_=y_ps[:], func=AF.Copy)
                nc.sync.dma_start(out=y_sl[rows, :], in_=y8[:])

            nc.gpsimd.collective_compute(
                "AllGather", OP.bypass,
                replica_groups=[list(range(n_cores))],
                ins=[y_sl[:]], outs=[ytab[:]],
            )

            # ============ Protein branch (fills AG(y) window) ============
            cvT_sb = cpool.tile([121, BL * 32], f16, tag="cvT")
            NG2 = BL // 2
            at_sbs = [None] * NG2

            def emit_at(g):
                at_ps = psm.tile([128, 512], f32, tag="mlp")
                for j in range(2):
                    b = 2 * g + j
                    oh = pool.tile([128, 8 * 32], f16, tag=f"oh{j}")
                    nc.vector.tensor_tensor(
                        out=oh[:].rearrange("p (k c) -> p k c", c=32),
                        in0=iota26_sb[:].rearrange("p (k c) -> p k c", c=32),
                        in1=t_sb[:, b * 8:(b + 1) * 8].unsqueeze(2)
                            .to_broadcast([128, 8, 32]),
                        op=OP.is_equal)
                    for ic in range(8):
                        icn = min(128, 1000 - ic * 128)
                        nc.tensor.matmul(out=at_ps[:32, 256 * j:256 * j + 256],
                                         lhsT=oh[:icn, ic * 32:(ic + 1) * 32],
                                         rhs=w2_sb[ic][:icn, :],
                                         start=(ic == 0), stop=(ic == 7))
                at_sb = pool.tile([32, 512], f16, tag=f"at_sb{g % 2}")
                nc.scalar.activation(out=at_sb[:], in_=at_ps[:32, :], func=AF.Copy)
                at_sbs[g] = at_sb

            def emit_cv(g):
                at_sb = at_sbs[g]
                cv_ps = psm.tile([128, 512], f32, tag="mlp")
                for j in range(2):
                    for k in range(8):
                        nc.tensor.matmul(
                            out=cv_ps[:121, 32 * j:32 * j + 32],
                            lhsT=embp4_sb[:32, k:k + 121],
                            rhs=at_sb[:32, 256 * j + k * 32:256 * j + (k + 1) * 32],
                            start=(k == 0), stop=(k == 7))
                nc.scalar.activation(out=cvT_sb[:, g * 64:(g + 1) * 64],
                                     in_=cv_ps[:121, :64], func=AF.Copy)

            emit_at(0)
            for g in range(NG2):
                if g + 1 < NG2:
                    emit_at(g + 1)
                emit_cv(g)
            xc1 = cpool.tile([128, BL], f16, tag="xc1")
            xt_ps = psm.tile([128, 512], f32, tag="mlp")
            for o in range(32):
                nc.tensor.matmul(
                    out=xt_ps[:, :BL], lhsT=fcxtp_sb[:121, o * 128:(o + 1) * 128],
                    rhs=cvT_sb[:].rearrange("p (b o) -> p b o", o=32)[:, :, o],
                    start=(o == 0), stop=(o == 31))
            nc.vector.tensor_scalar(out=xc1[:], in0=xt_ps[:, :BL],
                                    scalar1=fcxtb_sb[:, 0:1], scalar2=None,
                                    op0=OP.add)

            # ============ Phase D: GCN aggregation + pooling ============
            gap_acc = cpool.tile([BL, 7 * 128], f16, tag="gap_acc")
            gmpT_sb = []
            gapT_sb = []
            for kk, (ks, kn) in enumerate(FCH):
                gmpT_sb.append(cpool.tile([128, BL], f16, tag=f"gmpT{kk}",
                                          name=f"gmpT{kk}"))
                gapT_sb.append(cpool.tile([128, BL], f16, tag=f"gapT{kk}",
                                          name=f"gapT{kk}"))
            ng = 128 // GN
            for t in range(T):
                rows = slice(t * 128, (t + 1) * 128)
                cols = slice(t * K * 128, (t + 1) * K * 128)
                yg = pool.tile([128, K * HID], f8, tag="yg")
                for c in range(K):
                    nc.gpsimd.indirect_dma_start(
                        out=yg[:, c * HID:(c + 1) * HID],
                        out_offset=None, in_=ytab[:],
                        in_offset=bass.IndirectOffsetOnAxis(
                            ap=srcs_sb[:, t * K + c:t * K + c + 1], axis=0),
                    )
                wsel_sb = wpool.tile([128, K * 128], f16, tag="wsel", bufs=3)
                nc.sync.dma_start(out=wsel_sb[:], in_=wsel_t[:, cols])
                agg_ps = psg.tile([128, HID], f32, tag="g1")
                nc.tensor.matmul(out=agg_ps[:, :512], lhsT=ones1_sb[:],
                                 rhs=gcnb_sb[:, :512], start=True, stop=False)
                nc.tensor.matmul(out=agg_ps[:, 512:], lhsT=ones1_sb[:],
                                 rhs=gcnb_sb[:, 512:], start=True, stop=False)
                for c in range(K):
                    nc.tensor.matmul(out=agg_ps[:, :512],
                                     lhsT=wsel_sb[:, c * 128:(c + 1) * 128],
                                     rhs=yg[:, c * HID:c * HID + 512],
                                     start=False, stop=(c == K - 1))
                    nc.tensor.matmul(out=agg_ps[:, 512:],
                                     lhsT=wsel_sb[:, c * 128:(c + 1) * 128],
                                     rhs=yg[:, c * HID + 512:(c + 1) * HID],
                                     start=False, stop=(c == K - 1))
                g2b = pool.tile([128, 7 * 128], f16, tag="g2b")
                nc.scalar.activation(out=g2b[:, :HID], in_=agg_ps[:], func=AF.Relu)
                if dbg:
                    nc.sync.dma_start(out=o_g2[rows, :], in_=g2b[:, :HID])
                gp_ps = psg.tile([128, HID], f32, tag="g1")
                nc.tensor.matmul(out=gp_ps[:ng, :512], lhsT=poolm_sb[:, :ng],
                                 rhs=g2b[:, :512], start=True, stop=True)
                nc.tensor.matmul(out=gp_ps[:ng, 512:HID], lhsT=poolm_sb[:, :ng],
                                 rhs=g2b[:, 512:HID], start=True, stop=True)
                gtmp = pool.tile([ng, HID], f16, tag="gtmp")
                nc.scalar.activation(out=gtmp[:], in_=gp_ps[:ng, :HID], func=AF.Copy)
                nc.sync.dma_start(out=gap_acc[ng * t:ng * (t + 1), :HID], in_=gtmp[:])
                tpT = pool.tile([128, 7 * 128], f16, tag="tpT")
                nc.sync.dma_start_transpose(
                    out=tpT[:].rearrange("p (k d) -> p k d", d=128), in_=g2b[:])
                for kk, (ks, kn) in enumerate(FCH):
                    nc.vector.reduce_max(
                        out=gmpT_sb[kk][:kn, ng * t:ng * (t + 1)],
                        in_=tpT[:kn, kk * 128:(kk + 1) * 128]
                            .rearrange("p (g n) -> p g n", n=GN),
                        axis=AX.X)

            gtT = pool.tile([128, 7 * BL], f16, tag="gtT")
            nc.sync.dma_start_transpose(
                out=gtT[:].rearrange("p (k b) -> p k b", b=BL), in_=gap_acc[:])
            for kk, (ks, kn) in enumerate(FCH):
                nc.vector.tensor_copy(out=gapT_sb[kk][:kn, :],
                                      in_=gtT[:kn, kk * BL:(kk + 1) * BL])

            # ============ MLPs ============
            y1_sb = cpool.tile([128, 12 * BL], f16, tag="y1")
            mtiles = tiles(1500)
            for mj in range(0, len(mtiles), 2):
                y_ps = psm.tile([128, 512], f32, tag="mlp")
                for pj, (ms, mn) in enumerate(mtiles[mj:mj + 2]):
                    mi = mj + pj
                    col = pj * BL
                    wt_a = wpool.tile([128, 6 * 128], f16, tag=f"wta{pj}")
                    nc.sync.dma_start(
                        out=wt_a[:, :6 * mn].rearrange("p (k m) -> p k m", m=mn),
                        in_=fcg1_w[0:768, ms:ms + mn]
                            .rearrange("(k p) m -> p k m", p=128))
                    wt_b = wpool.tile([128, 128], f16, tag=f"wtb{pj}")
                    nc.sync.dma_start(out=wt_b[:12, :mn],
                                      in_=fcg1_w[768:780, ms:ms + mn])
                    wt_c = wpool.tile([128, 6 * 128], f16, tag=f"wtc{pj}")
                    nc.sync.dma_start(
                        out=wt_c[:, :6 * mn].rearrange("p (k m) -> p k m", m=mn),
                        in_=fcg1_w[HID:HID + 768, ms:ms + mn]
                            .rearrange("(k p) m -> p k m", p=128))
                    wt_d = wpool.tile([128, 128], f16, tag=f"wtd{pj}")
                    nc.sync.dma_start(out=wt_d[:12, :mn],
                                      in_=fcg1_w[HID + 768:HID + 780, ms:ms + mn])
                    for kk, (ks, kn) in enumerate(FCH):
                        lhsT = (wt_a[:, kk * mn:(kk + 1) * mn] if kk < 6
                                else wt_b[:12, :mn])
                        nc.tensor.matmul(out=y_ps[:mn, col:col + BL], lhsT=lhsT,
                                         rhs=gmpT_sb[kk][:kn, :],
                                         start=(kk == 0), stop=False)
                    for kk, (ks, kn) in enumerate(FCH):
                        lhsT = (wt_c[:, kk * mn:(kk + 1) * mn] if kk < 6
                                else wt_d[:12, :mn])
                        nc.tensor.matmul(out=y_ps[:mn, col:col + BL], lhsT=lhsT,
                                         rhs=gapT_sb[kk][:kn, :],
                                         start=False, stop=(kk == len(FCH) - 1))
                    nc.scalar.activation(out=y1_sb[:mn, mi * BL:(mi + 1) * BL],
                                         in_=y_ps[:mn, col:col + BL], func=AF.Relu,
                                         bias=fcg1b_sb[:mn, mi:mi + 1])

            xc0 = cpool.tile([128, BL], f16, tag="xc0")
            y2_ps = psm.tile([128, 512], f32, tag="mlp")
            kt2 = tiles(1500)
            for kk, (ks, kn) in enumerate(kt2):
                nc.tensor.matmul(out=y2_ps[:, :BL],
                                 lhsT=fcg2p_sb[:kn, kk * 128:kk * 128 + 128],
                                 rhs=y1_sb[:kn, kk * BL:(kk + 1) * BL],
                                 start=(kk == 0), stop=(kk == len(kt2) - 1))
            nc.vector.tensor_scalar(out=xc0[:], in0=y2_ps[:, :BL],
                                    scalar1=fcg2b_sb[:, 0:1], scalar2=None,
                                    op0=OP.add)

            # ---- head ----
            y3_sb = cpool.tile([128, 8 * BL], f16, tag="y3")
            for mi in range(8):
                y_ps = psm.tile([128, 512], f32, tag="mlp")
                for kk in range(2):
                    rhs = xc0 if kk == 0 else xc1
                    nc.tensor.matmul(
                        out=y_ps[:, :BL],
                        lhsT=fc1p_sb[:, kk * 1024 + mi * 128:
                                     kk * 1024 + (mi + 1) * 128],
                        rhs=rhs[:], start=(kk == 0), stop=(kk == 1))
                nc.scalar.activation(out=y3_sb[:, mi * BL:(mi + 1) * BL],
                                     in_=y_ps[:, :BL],
                                     func=AF.Relu, bias=fc1b_sb[:, mi:mi + 1])
            y4_sb = cpool.tile([128, 4 * BL], f16, tag="y4")
            for mi in range(4):
                y_ps = psm.tile([128, 512], f32, tag="mlp")
                for kk in range(8):
                    nc.tensor.matmul(out=y_ps[:, :BL],
                                     lhsT=fc2p_sb[:, kk * 512 + mi * 128:
                                                  kk * 512 + (mi + 1) * 128],
                                     rhs=y3_sb[:, kk * BL:(kk + 1) * BL],
                                     start=(kk == 0), stop=(kk == 7))
                nc.scalar.activation(out=y4_sb[:, mi * BL:(mi + 1) * BL],
                                     in_=y_ps[:, :BL],
                                     func=AF.Relu, bias=fc2b_sb[:, mi:mi + 1])
            o_ps = psm.tile([128, 512], f32, tag="mlp")
            for kk in range(4):
                nc.tensor.matmul(out=o_ps[:1, :BL], lhsT=outwp_sb[:, kk:kk + 1],
                                 rhs=y4_sb[:, kk * BL:(kk + 1) * BL],
                                 start=(kk == 0), stop=(kk == 3))
            o_sb = cpool.tile([1, BL], f32, tag="o_sb")
            nc.vector.tensor_scalar(out=o_sb[:], in0=o_ps[:1, :BL],
                                    scalar1=outb_sb[:, 0:1], scalar2=None,
                                    op0=OP.add)
            nc.sync.dma_start(out=outp[:], in_=o_sb[:])

    nc.finalize()
    return nc


def run(inp, n_cores=8, trace=False, dbg=False):
    from concourse.bass_utils import run_bass_kernel_spmd
    in_maps, cfg = host_prep(inp, n_cores)
    nc = build(cfg, dbg=dbg)
    res = run_bass_kernel_spmd(nc, in_maps, list(range(n_cores)))
    out = np.concatenate(
        [res.results[c]["outp"].reshape(-1, 1) for c in range(n_cores)], 0)
    return out, res


_CACHED = {}


def kernel(**inputs):
    """Full-input entry point: shards across 8 NeuronCores internally."""
    n_cores = 8
    in_maps, cfg = host_prep(inputs, n_cores)
    key = (cfg["N"], cfg["T"], cfg["BL"], cfg["K"], cfg["GN"])
    nc = _CACHED.get(key)
    if nc is None:
        nc = build(cfg)
        _CACHED[key] = nc
    from concourse.bass_utils import run_bass_kernel_spmd
    res = run_bass_kernel_spmd(nc, in_maps, list(range(n_cores)))
    out = np.concatenate(
        [res.results[c]["outp"].reshape(-1, 1) for c in range(n_cores)], 0)
    return out.astype(np.float32)


# revision 42
# speedup vs baseline: 1.1273x; 1.0333x over previous
"""GAT+GCN Trainium2 kernel: 8-core SPMD Bass/Tile implementation, v2.

Structure (per core):
  AG(x f16, 5MB) -> Phase B recomputes h[src]/a_src[src] per edge chunk from
  gathered x (contract dim 78), aggregates GAT messages (fh-major layout for
  DVE 2x), computes y = g1 @ gcn_w (f16, y-scale folded) -> y stored fp8,
  AllGathered in 4 row-chunks overlapping B -> Phase D gathers fp8 y,
  aggregates with host-precomputed wsel (sel*norm) tables, pools, MLPs.
  Protein branch (f16) scheduled into the AG(y) window.
"""
import numpy as np
import concourse.bass as bass
import concourse.bacc as bacc
import concourse.mybir as mybir
import concourse.tile as tile

f32 = mybir.dt.float32
f16 = mybir.dt.float16
f8 = mybir.dt.float8e4
i32 = mybir.dt.int32
AF = mybir.ActivationFunctionType
OP = mybir.AluOpType
AX = mybir.AxisListType

F = 78          # input feature dim
H = 10          # heads
HID = 780       # F*H
S_Y = 512.0     # fp8 scale for the y table (folded into gcn_w / fcg1_w)
NQ = 4          # y AllGather row chunks

F8NP = mybir.dt.np(f8)


def ceil_div(a, b):
    return (a + b - 1) // b


def host_prep(inp, n_cores=8):
    """Build per-core input maps + cfg from full inputs."""
    x = np.asarray(inp["x"], np.float32)
    ei = np.asarray(inp["edge_index"], np.int64)
    tgt = np.asarray(inp["target"], np.int64)
    N = x.shape[0]
    B = tgt.shape[0]
    GN = N // B                # nodes per graph
    NS = N // n_cores
    T = NS // 128
    BL = B // n_cores

    loops = np.arange(N, dtype=np.int64)
    src = np.concatenate([ei[0], loops])
    dst = np.concatenate([ei[1], loops])
    E = src.shape[0]

    deg = np.bincount(dst, minlength=N).astype(np.float64)
    dinv = 1.0 / np.sqrt(deg)
    normv = (dinv[src] * dinv[dst]).astype(np.float32)

    order = np.argsort(dst, kind="stable")
    src_s = src[order].astype(np.int32)
    dst_s = dst[order].astype(np.int32)
    norm_s = normv[order]

    gtile = dst_s // 128
    n_gtiles = N // 128
    starts = np.searchsorted(gtile, np.arange(n_gtiles))
    cnts = np.searchsorted(gtile, np.arange(n_gtiles), side="right") - starts
    dloc = (dst_s % 128).astype(np.int64)

    # appended self-loops (exactly one per node) go to chunk 0 at identity
    # slots so their gather degenerates to a contiguous local DMA
    is_loop = order >= ei.shape[1]
    nonloop = ~is_loop
    cum0 = np.concatenate([[0], np.cumsum(nonloop)])
    rank = cum0[np.arange(E)] - cum0[starts[gtile]]
    eslot = np.where(is_loop, dloc, rank % 128).astype(np.int64)
    chunk = np.where(is_loop, 0, rank // 128 + 1).astype(np.int64)
    K = 1 + int(np.max(ceil_div(np.maximum(cnts - 128, 0), 128)))

    srcs_p = np.zeros((n_gtiles, 128, K), np.int32)
    srcs_p[gtile, eslot, chunk] = src_s

    sel_d = np.zeros((n_gtiles, 128, K, 128), np.float16)
    sel_d[gtile, eslot, chunk, dloc] = 1.0
    wsel_d = np.zeros((n_gtiles, 128, K, 128), np.float16)
    wsel_d[gtile, eslot, chunk, dloc] = norm_s
    selT_d = sel_d.transpose(0, 3, 2, 1)          # [gt, d, k, e]

    gat_w = np.asarray(inp["gat_w"], np.float32)          # [F, H*F]
    att_src = np.asarray(inp["att_src"], np.float32)
    att_dst = np.asarray(inp["att_dst"], np.float32)
    As = np.einsum("fhc,hc->fh", gat_w.reshape(F, H, F), att_src)
    Ad = np.einsum("fhc,hc->fh", gat_w.reshape(F, H, F), att_dst)
    asad = np.concatenate([As, Ad], 1).astype(np.float16)     # [F, 2H]
    gatw_fh = np.ascontiguousarray(
        gat_w.reshape(F, H, F).transpose(0, 2, 1).reshape(F, HID)
    ).astype(np.float16)
    gat_b = np.asarray(inp["gat_b"], np.float32)
    gatb_fh = gat_b.reshape(H, F).T.ravel().reshape(1, HID).astype(np.float16)

    gcn_w = np.asarray(inp["gcn_w"], np.float32)
    gcnw_fh = np.ascontiguousarray(
        gcn_w.reshape(H, F, HID).transpose(1, 0, 2).reshape(HID, HID)
    ) * S_Y
    gcnw_fh = gcnw_fh.astype(np.float16)
    gcnbS = (np.asarray(inp["gcn_b"], np.float32) * S_Y).reshape(1, HID)
    gcnbS = gcnbS.astype(np.float16)
    ones1 = np.ones((1, 128), np.float16)

    fcg1_w = np.asarray(inp["fcg1_w"], np.float32).copy()
    fcg1_w *= 1.0 / S_Y
    fcg1_w[HID:] *= 1.0 / GN
    fcg1_w = fcg1_w.astype(np.float16)

    def bias_sw(b, mt):
        b = np.asarray(b, np.float32)
        out = np.zeros((mt * 128,), np.float32)
        out[: b.shape[0]] = b
        return np.ascontiguousarray(out.reshape(mt, 128).T)

    fcg1_bsw = bias_sw(inp["fcg1_b"], 12)
    fcg2_w = np.asarray(inp["fcg2_w"], np.float32)
    fcg2_bsw = bias_sw(inp["fcg2_b"], 1)
    fcg2p = np.zeros((128, 12 * 128), np.float16)
    for kk in range(12):
        kn = min(128, 1500 - kk * 128)
        fcg2p[:kn, kk * 128:kk * 128 + 128] = fcg2_w[kk * 128:kk * 128 + kn]

    convxt_w = np.asarray(inp["convxt_w"], np.float32)
    W2 = np.ascontiguousarray(
        convxt_w.transpose(1, 2, 0).reshape(1000, 8 * 32)).astype(np.float16)
    emb = np.asarray(inp["emb"], np.float32)
    fcxt_w = np.asarray(inp["fcxt_w"], np.float32)
    cb = np.asarray(inp["convxt_b"], np.float32)
    bias_fold = (cb[:, None] * fcxt_w.reshape(32, 121, 128).sum(1)).sum(0)
    fcxt_bsw = bias_sw(np.asarray(inp["fcxt_b"], np.float32) + bias_fold, 1)
    fcxtp = np.zeros((128, 32 * 128), np.float16)
    for o in range(32):
        fcxtp[:121, o * 128:(o + 1) * 128] = fcxt_w[o * 121:(o + 1) * 121]

    fc1_w = np.asarray(inp["fc1_w"], np.float32)
    fc1_bsw = bias_sw(inp["fc1_b"], 8)
    fc1p = np.ascontiguousarray(
        fc1_w.reshape(2, 128, 1024).transpose(1, 0, 2).reshape(128, 2048)
    ).astype(np.float16)
    fc2_w = np.asarray(inp["fc2_w"], np.float32)
    fc2_bsw = bias_sw(inp["fc2_b"], 4)
    fc2p = np.ascontiguousarray(
        fc2_w.reshape(8, 128, 512).transpose(1, 0, 2).reshape(128, 8 * 512)
    ).astype(np.float16)
    out_w = np.asarray(inp["out_w"], np.float32)
    outwp = np.ascontiguousarray(out_w.reshape(4, 128).T).astype(np.float16)
    out_b = np.asarray(inp["out_b"], np.float32).reshape(1, 1)

    iota26 = np.broadcast_to(
        np.tile(np.arange(32, dtype=np.float16), 8), (128, 8 * 32)).copy()
    embp4 = np.zeros((128, 128), np.float16)
    for b4 in range(4):
        embp4[b4 * 32:b4 * 32 + 26, :] = emb
    ng = 128 // GN
    poolm = np.zeros((128, ng), np.float16)
    for g in range(ng):
        poolm[g * GN:(g + 1) * GN, g] = 1.0

    shared = dict(
        asad=asad, gatw_fh=gatw_fh, gatb_fh=gatb_fh, gcnw_fh=gcnw_fh,
        gcnbS=gcnbS, ones1=ones1, fcg1_w=fcg1_w, fcg1_bsw=fcg1_bsw,
        fcg2p=fcg2p, fcg2_bsw=fcg2_bsw, w2=W2, embp4=embp4, iota26=iota26,
        fcxtp=fcxtp, fcxt_bsw=fcxt_bsw, fc1p=fc1p, fc1_bsw=fc1_bsw,
        fc2p=fc2p, fc2_bsw=fc2_bsw, outwp=outwp, out_b=out_b,
        poolm=poolm,
    )

    x16 = np.ascontiguousarray(x).astype(np.float16)
    in_maps = []
    for c in range(n_cores):
        m = dict(shared)
        m["x_full"] = x16
        m["x_sl"] = np.ascontiguousarray(x[c * NS:(c + 1) * NS]).astype(np.float16)
        gt = slice(c * T, (c + 1) * T)
        m["srcs"] = np.ascontiguousarray(
            srcs_p[gt].transpose(1, 0, 2).reshape(128, T * K))
        m["sel_t"] = np.ascontiguousarray(
            sel_d[gt].transpose(1, 0, 2, 3).reshape(128, T * K * 128))
        m["selT_t"] = np.ascontiguousarray(
            selT_d[gt].transpose(1, 0, 2, 3).reshape(128, T * K * 128))
        m["wsel_t"] = np.ascontiguousarray(
            wsel_d[gt].transpose(1, 0, 2, 3).reshape(128, T * K * 128))
        tpad = np.zeros((BL, 1024), np.int64)
        tpad[:, :1000] = tgt[c * BL:(c + 1) * BL]
        tl = tpad.reshape(BL, 8, 128)
        m["t_sb"] = np.ascontiguousarray(
            tl.transpose(2, 0, 1).reshape(128, BL * 8).astype(np.float16))
        in_maps.append(m)

    cfg = dict(n_cores=n_cores, N=N, NS=NS, T=T, BL=BL, K=K, GN=GN)
    return in_maps, cfg


def build(cfg, dbg=False):
    n_cores, NS, T, BL, K, GN = (cfg["n_cores"], cfg["NS"], cfg["T"],
                                 cfg["BL"], cfg["K"], cfg["GN"])
    N = cfg["N"]

    nc = bacc.Bacc(None, target_bir_lowering=False)

    def dinp(name, shape, dt=f32):
        return nc.dram_tensor(name, list(shape), dt, kind="ExternalInput")

    x_sl = dinp("x_sl", (NS, F), f16)
    srcs = dinp("srcs", (128, T * K), i32)
    sel_t = dinp("sel_t", (128, T * K * 128), f16)
    selT_t = dinp("selT_t", (128, T * K * 128), f16)
    wsel_t = dinp("wsel_t", (128, T * K * 128), f16)
    t_sb_d = dinp("t_sb", (128, BL * 8), f16)
    asad_d = dinp("asad", (F, 2 * H), f16)
    gatw_d = dinp("gatw_fh", (F, HID), f16)
    gatb_d = dinp("gatb_fh", (1, HID), f16)
    gcnw_d = dinp("gcnw_fh", (HID, HID), f16)
    gcnb_d = dinp("gcnbS", (1, HID), f16)
    ones1_d = dinp("ones1", (1, 128), f16)
    fcg1_w = dinp("fcg1_w", (2 * HID, 1500), f16)
    fcg1_bsw = dinp("fcg1_bsw", (128, 12))
    fcg2p_d = dinp("fcg2p", (128, 12 * 128), f16)
    fcg2_bsw = dinp("fcg2_bsw", (128, 1))
    w2_d = dinp("w2", (1000, 256), f16)
    embp4_d = dinp("embp4", (128, 128), f16)
    iota26_d = dinp("iota26", (128, 8 * 32), f16)
    fcxtp_d = dinp("fcxtp", (128, 32 * 128), f16)
    fcxt_bsw = dinp("fcxt_bsw", (128, 1))
    fc1p_d = dinp("fc1p", (128, 2048), f16)
    fc1_bsw = dinp("fc1_bsw", (128, 8))
    fc2p_d = dinp("fc2p", (128, 8 * 512), f16)
    fc2_bsw = dinp("fc2_bsw", (128, 4))
    outwp_d = dinp("outwp", (128, 4), f16)
    out_b = dinp("out_b", (1, 1))
    poolm_d = dinp("poolm", (128, 128 // GN), f16)

    outp = nc.dram_tensor("outp", [1, BL], f32, kind="ExternalOutput")
    if dbg:
        o_g1 = nc.dram_tensor("o_g1", [NS, HID], f16, kind="ExternalOutput")
        o_y = nc.dram_tensor("o_y", [NS, HID], f32, kind="ExternalOutput")
        o_g2 = nc.dram_tensor("o_g2", [NS, HID], f16, kind="ExternalOutput")

    xtab = dinp("x_full", (N, F), f16)
    y_sl = nc.dram_tensor("y_sl", [NS, HID], f8)
    ytab = nc.dram_tensor("ytab", [N, HID], f8, addr_space="Shared")

    FCH = [(kk * 128, min(128, HID - kk * 128)) for kk in range(ceil_div(HID, 128))]

    def tiles(n, step=128):
        return [(s, min(step, n - s)) for s in range(0, n, step)]

    with tile.TileContext(nc) as tc:
        with (
            tc.tile_pool(name="const", bufs=1) as cpool,
            tc.tile_pool(name="sb", bufs=3) as pool,
            tc.tile_pool(name="w", bufs=2) as wpool,
            tc.tile_pool(name="ps", bufs=2, space="PSUM") as psp,
            tc.tile_pool(name="psg", bufs=1, space="PSUM") as psg,
            tc.tile_pool(name="psm", bufs=2, space="PSUM") as psm,
        ):
            # ---------- resident constants ----------
            def load_const(name, dram, shape, dt=f32):
                t_ = cpool.tile(list(shape), dt, tag=name, name=name)
                nc.sync.dma_start(out=t_[:], in_=dram[:])
                return t_

            asad_sb = load_const("asad", asad_d, [F, 2 * H], f16)
            gatw_sb = load_const("gatw", gatw_d, [F, HID], f16)
            gatb_sb = load_const("gatb", gatb_d, [1, HID], f16)
            gcnb_sb = load_const("gcnb", gcnb_d, [1, HID], f16)
            ones1_sb = load_const("ones1", ones1_d, [1, 128], f16)
            poolm_sb = load_const("poolm", poolm_d, [128, 2], f16)
            t_sb = load_const("tsb", t_sb_d, [128, BL * 8], f16)
            iota26_sb = load_const("iota26", iota26_d, [128, 8 * 32], f16)
            embp4_sb = load_const("embp4", embp4_d, [128, 128], f16)
            fcxtp_sb = load_const("fcxtp", fcxtp_d, [128, 32 * 128], f16)
            fcg1b_sb = load_const("fcg1b", fcg1_bsw, [128, 12])
            fcg2p_sb = load_const("fcg2p", fcg2p_d, [128, 12 * 128], f16)
            fcg2b_sb = load_const("fcg2b", fcg2_bsw, [128, 1])
            fcxtb_sb = load_const("fcxtb", fcxt_bsw, [128, 1])
            fc1p_sb = load_const("fc1p", fc1p_d, [128, 2048], f16)
            fc1b_sb = load_const("fc1b", fc1_bsw, [128, 8])
            fc2p_sb = load_const("fc2p", fc2p_d, [128, 8 * 512], f16)
            fc2b_sb = load_const("fc2b", fc2_bsw, [128, 4])
            outwp_sb = load_const("outwp", outwp_d, [128, 4], f16)
            outb_sb = load_const("outb", out_b, [1, 1])
            srcs_sb = load_const("srcs_all", srcs, [128, T * K], i32)
            selTall_sb = load_const("selT_all", selT_t, [128, T * K * 128], f16)
            gcnw_sb = []
            for kk, (ks, kn) in enumerate(FCH):
                t_ = cpool.tile([128, HID], f16, tag=f"gcnw{kk}", name=f"gcnw{kk}")
                nc.sync.dma_start(out=t_[:kn, :], in_=gcnw_d[ks:ks + kn, :])
                gcnw_sb.append(t_)
            w2_sb = []
            for ic in range(8):
                icn = min(128, 1000 - ic * 128)
                t_ = cpool.tile([128, 256], f16, tag=f"w2{ic}", name=f"w2{ic}")
                nc.sync.dma_start(out=t_[:icn, :], in_=w2_d[ic * 128:ic * 128 + icn, :])
                w2_sb.append(t_)
            adst_sb = cpool.tile([128, T * H], f16, tag="adst")

            # ============ Phase A': a_dst for local nodes ============
            for t in range(T):
                rows = slice(t * 128, (t + 1) * 128)
                x_t = pool.tile([128, 128], f16, tag="x_t")
                nc.sync.dma_start(out=x_t[:, :F], in_=x_sl[rows, :])
                xT = pool.tile([128, 128], f16, tag="xT")
                nc.sync.dma_start_transpose(out=xT[:], in_=x_t[:])
                ad_ps = psm.tile([128, 512], f32, tag="mlp")
                nc.tensor.matmul(out=ad_ps[:, :H], lhsT=xT[:F, :],
                                 rhs=asad_sb[:, H:2 * H], start=True, stop=True)
                nc.vector.tensor_copy(out=adst_sb[:, t * H:(t + 1) * H],
                                      in_=ad_ps[:, :H])

            # ============ Phase B': GAT + y per tile ============
            for t in range(T):
                rows = slice(t * 128, (t + 1) * 128)
                cols = slice(t * K * 128, (t + 1) * K * 128)
                xg = pool.tile([128, K * 128], f16, tag="xg")
                sel_sb = wpool.tile([128, K * 128], f16, tag="sel")
                nc.sync.dma_start(out=sel_sb[:], in_=sel_t[:, cols])
                nc.sync.dma_start(out=xg[:, 0:F], in_=x_sl[rows, :])
                for c in range(1, K):
                    nc.gpsimd.indirect_dma_start(
                        out=xg[:, c * 128:c * 128 + F],
                        out_offset=None, in_=xtab[:],
                        in_offset=bass.IndirectOffsetOnAxis(
                            ap=srcs_sb[:, t * K + c:t * K + c + 1], axis=0),
                    )

                # pass 1: transposed x chunks, a_src, a_dst, denominators
                asm_ps = psm.tile([128, 512], f32, tag="mlp")
                xcT = pool.tile([128, K * 128], f16, tag="xcT")
                nc.sync.dma_start_transpose(
                    out=xcT[:].rearrange("p (k d) -> p k d", d=128), in_=xg[:])
                for c in range(K):
                    nc.tensor.matmul(out=asm_ps[:, c * H:(c + 1) * H],
                                     lhsT=xcT[:F, c * 128:(c + 1) * 128],
                                     rhs=asad_sb[:, :H],
                                     start=True, stop=False)
                    nc.tensor.matmul(out=asm_ps[:, c * H:(c + 1) * H],
                                     lhsT=selTall_sb[:, t * K * 128 + c * 128:t * K * 128 + (c + 1) * 128],
                                     rhs=adst_sb[:, t * H:(t + 1) * H],
                                     start=False, stop=True)
                al2 = pool.tile([128, K * H], f32, tag="al2")
                nc.vector.tensor_scalar(out=al2[:], in0=asm_ps[:, :K * H],
                                        scalar1=0.2, scalar2=None, op0=OP.mult)
                nc.vector.tensor_tensor(out=al2[:], in0=al2[:],
                                        in1=asm_ps[:, :K * H], op=OP.max)
                p16 = pool.tile([128, K * H], f16, tag="p16")
                nc.scalar.activation(out=p16[:], in_=al2[:], func=AF.Exp)
                for c in range(K):
                    nc.tensor.matmul(out=asm_ps[:, 192:192 + H],
                                     lhsT=sel_sb[:, c * 128:(c + 1) * 128],
                                     rhs=p16[:, c * H:(c + 1) * H],
                                     start=(c == 0), stop=(c == K - 1))
                rd = pool.tile([128, H], f16, tag="rd")
                with nc.allow_low_precision(reason="rd f16 as matmul rhs"):
                    nc.vector.reciprocal(out=rd[:], in_=asm_ps[:, 192:192 + H])

                # per-edge 1/denom via selT gather-matmul; fold into p16
                rde_ps = psm.tile([128, 512], f32, tag="mlp")
                for c in range(K):
                    nc.tensor.matmul(out=rde_ps[:, c * H:(c + 1) * H],
                                     lhsT=selTall_sb[:, t * K * 128 + c * 128:t * K * 128 + (c + 1) * 128],
                                     rhs=rd[:], start=True, stop=True)
                p16q = pool.tile([128, K * H], f16, tag="p16q")
                nc.vector.tensor_tensor(out=p16q[:], in0=p16[:],
                                        in1=rde_ps[:, :K * H], op=OP.mult)

                # pass 2: h recompute, messages, normalized aggregation
                h_tiles = [None] * K

                def emit_h(c):
                    hp = psp.tile([128, HID], f32, tag="h")
                    nc.tensor.matmul(out=hp[:, :512],
                                     lhsT=xcT[:F, c * 128:(c + 1) * 128],
                                     rhs=gatw_sb[:, :512], start=True, stop=True)
                    nc.tensor.matmul(out=hp[:, 512:],
                                     lhsT=xcT[:F, c * 128:(c + 1) * 128],
                                     rhs=gatw_sb[:, 512:], start=True, stop=True)
                    h_tiles[c] = hp

                emit_h(0)
                emit_h(1)
                g1_ps = psg.tile([128, HID], f32, tag="g1")
                nc.tensor.matmul(out=g1_ps[:, :512], lhsT=ones1_sb[:],
                                 rhs=gatb_sb[:, :512], start=True, stop=False)
                nc.tensor.matmul(out=g1_ps[:, 512:], lhsT=ones1_sb[:],
                                 rhs=gatb_sb[:, 512:], start=True, stop=False)
                for c in range(K):
                    if c + 2 < K:
                        emit_h(c + 2)
                    h_ps = h_tiles[c]
                    m = pool.tile([128, HID], f16, tag=f"m{c % 2}")
                    if c % 2 == 0:
                        nc.vector.tensor_tensor(
                            out=m[:].rearrange("p (f h) -> p f h", h=H),
                            in0=h_ps[:].rearrange("p (f h) -> p f h", h=H),
                            in1=p16q[:, c * H:(c + 1) * H].unsqueeze(1)
                                .to_broadcast([128, F, H]),
                            op=OP.mult)
                    else:
                        h16 = pool.tile([128, HID], f16, tag=f"h16{c % 2}")
                        nc.scalar.activation(out=h16[:], in_=h_ps[:], func=AF.Copy)
                        nc.vector.tensor_tensor(
                            out=m[:].rearrange("p (f h) -> p f h", h=H),
                            in0=h16[:].rearrange("p (f h) -> p f h", h=H),
                            in1=p16q[:, c * H:(c + 1) * H].unsqueeze(1)
                                .to_broadcast([128, F, H]),
                            op=OP.mult)
                    selc = sel_sb[:, c * 128:(c + 1) * 128]
                    nc.tensor.matmul(out=g1_ps[:, :512], lhsT=selc,
                                     rhs=m[:, :512],
                                     start=False, stop=(c == K - 1))
                    nc.tensor.matmul(out=g1_ps[:, 512:], lhsT=selc,
                                     rhs=m[:, 512:],
                                     start=False, stop=(c == K - 1))
                g1t = pool.tile([128, 7 * 128], f16, tag="g1t")
                nc.scalar.activation(out=g1t[:, :HID], in_=g1_ps[:], func=AF.Relu)
                if dbg:
                    nc.sync.dma_start(out=o_g1[rows, :], in_=g1t[:, :HID])

                # y = g1 @ gcn_w  (f16, S_Y folded into gcnw)
                y_ps = psg.tile([128, HID], f32, tag="g1")
                gT = pool.tile([128, 7 * 128], f16, tag="gT")
                nc.sync.dma_start_transpose(
                    out=gT[:].rearrange("p (k d) -> p k d", d=128), in_=g1t[:])
                for kk, (ks, kn) in enumerate(FCH):
                    nc.tensor.matmul(out=y_ps[:, :512],
                                     lhsT=gT[:kn, kk * 128:(kk + 1) * 128],
                                     rhs=gcnw_sb[kk][:kn, :512],
                                     start=(kk == 0), stop=(kk == len(FCH) - 1))
                    nc.tensor.matmul(out=y_ps[:, 512:],
                                     lhsT=gT[:kn, kk * 128:(kk + 1) * 128],
                                     rhs=gcnw_sb[kk][:kn, 512:],
                                     start=(kk == 0), stop=(kk == len(FCH) - 1))
                if dbg:
                    ydbg = pool.tile([128, HID], f32, tag="ydbg")
                    nc.scalar.activation(out=ydbg[:], in_=y_ps[:], func=AF.Copy)
                    nc.sync.dma_start(out=o_y[rows, :], in_=ydbg[:])
                y8 = pool.tile([128, HID], f8, tag="y8")
                nc.scalar.activation(out=y8[:], in_=y_ps[:], func=AF.Copy)
                nc.sync.dma_start(out=y_sl[rows, :], in_=y8[:])

            nc.gpsimd.collective_compute(
                "AllGather", OP.bypass,
                replica_groups=[list(range(n_cores))],
                ins=[y_sl[:]], outs=[ytab[:]],
            )

            # ============ Protein branch (fills AG(y) window) ============
            cvT_sb = cpool.tile([121, BL * 32], f16, tag="cvT")
            NG2 = BL // 2
            at_sbs = [None] * NG2

            def emit_at(g):
                at_ps = psm.tile([128, 512], f32, tag="mlp")
                for j in range(2):
                    b = 2 * g + j
                    oh = pool.tile([128, 8 * 32], f16, tag=f"oh{j}")
                    nc.vector.tensor_tensor(
                        out=oh[:].rearrange("p (k c) -> p k c", c=32),
                        in0=iota26_sb[:].rearrange("p (k c) -> p k c", c=32),
                        in1=t_sb[:, b * 8:(b + 1) * 8].unsqueeze(2)
                            .to_broadcast([128, 8, 32]),
                        op=OP.is_equal)
                    for ic in range(8):
                        icn = min(128, 1000 - ic * 128)
                        nc.tensor.matmul(out=at_ps[:32, 256 * j:256 * j + 256],
                                         lhsT=oh[:icn, ic * 32:(ic + 1) * 32],
                                         rhs=w2_sb[ic][:icn, :],
                                         start=(ic == 0), stop=(ic == 7))
                at_sb = pool.tile([32, 512], f16, tag=f"at_sb{g % 2}")
                nc.scalar.activation(out=at_sb[:], in_=at_ps[:32, :], func=AF.Copy)
                at_sbs[g] = at_sb

            def emit_cv(g):
                at_sb = at_sbs[g]
                cv_ps = psm.tile([128, 512], f32, tag="mlp")
                for j in range(2):
                    for k in range(8):
                        nc.tensor.matmul(
                            out=cv_ps[:121, 32 * j:32 * j + 32],
                            lhsT=embp4_sb[:32, k:k + 121],
                            rhs=at_sb[:32, 256 * j + k * 32:256 * j + (k + 1) * 32],
                            start=(k == 0), stop=(k == 7))
                nc.scalar.activation(out=cvT_sb[:, g * 64:(g + 1) * 64],
                                     in_=cv_ps[:121, :64], func=AF.Copy)

            emit_at(0)
            for g in range(NG2):
                if g + 1 < NG2:
                    emit_at(g + 1)
                emit_cv(g)
            xc1 = cpool.tile([128, BL], f16, tag="xc1")
            xt_ps = psm.tile([128, 512], f32, tag="mlp")
            for o in range(32):
                nc.tensor.matmul(
                    out=xt_ps[:, :BL], lhsT=fcxtp_sb[:121, o * 128:(o + 1) * 128],
                    rhs=cvT_sb[:].rearrange("p (b o) -> p b o", o=32)[:, :, o],
                    start=(o == 0), stop=(o == 31))
            nc.vector.tensor_scalar(out=xc1[:], in0=xt_ps[:, :BL],
                                    scalar1=fcxtb_sb[:, 0:1], scalar2=None,
                                    op0=OP.add)

            # ============ Phase D: GCN aggregation + pooling ============
            gap_acc = cpool.tile([BL, 7 * 128], f16, tag="gap_acc")
            gmpT_sb = []
            gapT_sb = []
            for kk, (ks, kn) in enumerate(FCH):
                gmpT_sb.append(cpool.tile([128, BL], f16, tag=f"gmpT{kk}",
                                          name=f"gmpT{kk}"))
                gapT_sb.append(cpool.tile([128, BL], f16, tag=f"gapT{kk}",
                                          name=f"gapT{kk}"))
            ng = 128 // GN
            for t in range(T):
                rows = slice(t * 128, (t + 1) * 128)
                cols = slice(t * K * 128, (t + 1) * K * 128)
                yg = pool.tile([128, K * HID], f8, tag="yg")
                nc.sync.dma_start(out=yg[:, 0:HID], in_=y_sl[rows, :])
                for c in range(1, K):
                    nc.gpsimd.indirect_dma_start(
                        out=yg[:, c * HID:(c + 1) * HID],
                        out_offset=None, in_=ytab[:],
                        in_offset=bass.IndirectOffsetOnAxis(
                            ap=srcs_sb[:, t * K + c:t * K + c + 1], axis=0),
                    )
                wsel_sb = wpool.tile([128, K * 128], f16, tag="wsel", bufs=3)
                nc.sync.dma_start(out=wsel_sb[:], in_=wsel_t[:, cols])
                agg_ps = psg.tile([128, HID], f32, tag="g1")
                nc.tensor.matmul(out=agg_ps[:, :512], lhsT=ones1_sb[:],
                                 rhs=gcnb_sb[:, :512], start=True, stop=False)
                nc.tensor.matmul(out=agg_ps[:, 512:], lhsT=ones1_sb[:],
                                 rhs=gcnb_sb[:, 512:], start=True, stop=False)
                for c in range(K):
                    nc.tensor.matmul(out=agg_ps[:, :512],
                                     lhsT=wsel_sb[:, c * 128:(c + 1) * 128],
                                     rhs=yg[:, c * HID:c * HID + 512],
                                     start=False, stop=(c == K - 1))
                    nc.tensor.matmul(out=agg_ps[:, 512:],
                                     lhsT=wsel_sb[:, c * 128:(c + 1) * 128],
                                     rhs=yg[:, c * HID + 512:(c + 1) * HID],
                                     start=False, stop=(c == K - 1))
                g2b = pool.tile([128, 7 * 128], f16, tag="g2b")
                nc.scalar.activation(out=g2b[:, :HID], in_=agg_ps[:], func=AF.Relu)
                if dbg:
                    nc.sync.dma_start(out=o_g2[rows, :], in_=g2b[:, :HID])
                gp_ps = psg.tile([128, HID], f32, tag="g1")
                nc.tensor.matmul(out=gp_ps[:ng, :512], lhsT=poolm_sb[:, :ng],
                                 rhs=g2b[:, :512], start=True, stop=True)
                nc.tensor.matmul(out=gp_ps[:ng, 512:HID], lhsT=poolm_sb[:, :ng],
                                 rhs=g2b[:, 512:HID], start=True, stop=True)
                gtmp = pool.tile([ng, HID], f16, tag="gtmp")
                nc.scalar.activation(out=gtmp[:], in_=gp_ps[:ng, :HID], func=AF.Copy)
                nc.sync.dma_start(out=gap_acc[ng * t:ng * (t + 1), :HID], in_=gtmp[:])
                tpT = pool.tile([128, 7 * 128], f16, tag="tpT")
                nc.sync.dma_start_transpose(
                    out=tpT[:].rearrange("p (k d) -> p k d", d=128), in_=g2b[:])
                for kk, (ks, kn) in enumerate(FCH):
                    nc.vector.reduce_max(
                        out=gmpT_sb[kk][:kn, ng * t:ng * (t + 1)],
                        in_=tpT[:kn, kk * 128:(kk + 1) * 128]
                            .rearrange("p (g n) -> p g n", n=GN),
                        axis=AX.X)

            gtT = pool.tile([128, 7 * BL], f16, tag="gtT")
            nc.sync.dma_start_transpose(
                out=gtT[:].rearrange("p (k b) -> p k b", b=BL), in_=gap_acc[:])
            for kk, (ks, kn) in enumerate(FCH):
                nc.vector.tensor_copy(out=gapT_sb[kk][:kn, :],
                                      in_=gtT[:kn, kk * BL:(kk + 1) * BL])

            # ============ MLPs ============
            y1_sb = cpool.tile([128, 12 * BL], f16, tag="y1")
            mtiles = tiles(1500)
            for mj in range(0, len(mtiles), 2):
                y_ps = psm.tile([128, 512], f32, tag="mlp")
                for pj, (ms, mn) in enumerate(mtiles[mj:mj + 2]):
                    mi = mj + pj
                    col = pj * BL
                    wt_a = wpool.tile([128, 6 * 128], f16, tag=f"wta{pj}")
                    nc.sync.dma_start(
                        out=wt_a[:, :6 * mn].rearrange("p (k m) -> p k m", m=mn),
                        in_=fcg1_w[0:768, ms:ms + mn]
                            .rearrange("(k p) m -> p k m", p=128))
                    wt_b = wpool.tile([128, 128], f16, tag=f"wtb{pj}")
                    nc.sync.dma_start(out=wt_b[:12, :mn],
                                      in_=fcg1_w[768:780, ms:ms + mn])
                    wt_c = wpool.tile([128, 6 * 128], f16, tag=f"wtc{pj}")
                    nc.sync.dma_start(
                        out=wt_c[:, :6 * mn].rearrange("p (k m) -> p k m", m=mn),
                        in_=fcg1_w[HID:HID + 768, ms:ms + mn]
                            .rearrange("(k p) m -> p k m", p=128))
                    wt_d = wpool.tile([128, 128], f16, tag=f"wtd{pj}")
                    nc.sync.dma_start(out=wt_d[:12, :mn],
                                      in_=fcg1_w[HID + 768:HID + 780, ms:ms + mn])
                    for kk, (ks, kn) in enumerate(FCH):
                        lhsT = (wt_a[:, kk * mn:(kk + 1) * mn] if kk < 6
                                else wt_b[:12, :mn])
                        nc.tensor.matmul(out=y_ps[:mn, col:col + BL], lhsT=lhsT,
                                         rhs=gmpT_sb[kk][:kn, :],
                                         start=(kk == 0), stop=False)
                    for kk, (ks, kn) in enumerate(FCH):
                        lhsT = (wt_c[:, kk * mn:(kk + 1) * mn] if kk < 6
                                else wt_d[:12, :mn])
                        nc.tensor.matmul(out=y_ps[:mn, col:col + BL], lhsT=lhsT,
                                         rhs=gapT_sb[kk][:kn, :],
                                         start=False, stop=(kk == len(FCH) - 1))
                    nc.scalar.activation(out=y1_sb[:mn, mi * BL:(mi + 1) * BL],
                                         in_=y_ps[:mn, col:col + BL], func=AF.Relu,
                                         bias=fcg1b_sb[:mn, mi:mi + 1])

            xc0 = cpool.tile([128, BL], f16, tag="xc0")
            y2_ps = psm.tile([128, 512], f32, tag="mlp")
            kt2 = tiles(1500)
            for kk, (ks, kn) in enumerate(kt2):
                nc.tensor.matmul(out=y2_ps[:, :BL],
                                 lhsT=fcg2p_sb[:kn, kk * 128:kk * 128 + 128],
                                 rhs=y1_sb[:kn, kk * BL:(kk + 1) * BL],
                                 start=(kk == 0), stop=(kk == len(kt2) - 1))
            nc.vector.tensor_scalar(out=xc0[:], in0=y2_ps[:, :BL],
                                    scalar1=fcg2b_sb[:, 0:1], scalar2=None,
                                    op0=OP.add)

            # ---- head ----
            y3_sb = cpool.tile([128, 8 * BL], f16, tag="y3")
            for mi in range(8):
                y_ps = psm.tile([128, 512], f32, tag="mlp")
                for kk in range(2):
                    rhs = xc0 if kk == 0 else xc1
                    nc.tensor.matmul(
                        out=y_ps[:, :BL],
                        lhsT=fc1p_sb[:, kk * 1024 + mi * 128:
                                     kk * 1024 + (mi + 1) * 128],
                        rhs=rhs[:], start=(kk == 0), stop=(kk == 1))
                nc.scalar.activation(out=y3_sb[:, mi * BL:(mi + 1) * BL],
                                     in_=y_ps[:, :BL],
                                     func=AF.Relu, bias=fc1b_sb[:, mi:mi + 1])
            y4_sb = cpool.tile([128, 4 * BL], f16, tag="y4")
            for mi in range(4):
                y_ps = psm.tile([128, 512], f32, tag="mlp")
                for kk in range(8):
                    nc.tensor.matmul(out=y_ps[:, :BL],
                                     lhsT=fc2p_sb[:, kk * 512 + mi * 128:
                                                  kk * 512 + (mi + 1) * 128],
                                     rhs=y3_sb[:, kk * BL:(kk + 1) * BL],
                                     start=(kk == 0), stop=(kk == 7))
                nc.scalar.activation(out=y4_sb[:, mi * BL:(mi + 1) * BL],
                                     in_=y_ps[:, :BL],
                                     func=AF.Relu, bias=fc2b_sb[:, mi:mi + 1])
            o_ps = psm.tile([128, 512], f32, tag="mlp")
            for kk in range(4):
                nc.tensor.matmul(out=o_ps[:1, :BL], lhsT=outwp_sb[:, kk:kk + 1],
                                 rhs=y4_sb[:, kk * BL:(kk + 1) * BL],
                                 start=(kk == 0), stop=(kk == 3))
            o_sb = cpool.tile([1, BL], f32, tag="o_sb")
            nc.vector.tensor_scalar(out=o_sb[:], in0=o_ps[:1, :BL],
                                    scalar1=outb_sb[:, 0:1], scalar2=None,
                                    op0=OP.add)
            nc.sync.dma_start(out=outp[:], in_=o_sb[:])

    nc.finalize()
    return nc


def run(inp, n_cores=8, trace=False, dbg=False):
    from concourse.bass_utils import run_bass_kernel_spmd
    in_maps, cfg = host_prep(inp, n_cores)
    nc = build(cfg, dbg=dbg)
    res = run_bass_kernel_spmd(nc, in_maps, list(range(n_cores)))
    out = np.concatenate(
        [res.results[c]["outp"].reshape(-1, 1) for c in range(n_cores)], 0)
    return out, res


_CACHED = {}


def kernel(**inputs):
    """Full-input entry point: shards across 8 NeuronCores internally."""
    n_cores = 8
    in_maps, cfg = host_prep(inputs, n_cores)
    key = (cfg["N"], cfg["T"], cfg["BL"], cfg["K"], cfg["GN"])
    nc = _CACHED.get(key)
    if nc is None:
        nc = build(cfg)
        _CACHED[key] = nc
    from concourse.bass_utils import run_bass_kernel_spmd
    res = run_bass_kernel_spmd(nc, in_maps, list(range(n_cores)))
    out = np.concatenate(
        [res.results[c]["outp"].reshape(-1, 1) for c in range(n_cores)], 0)
    return out.astype(np.float32)
